# revision 5
# baseline (speedup 1.0000x reference)
"""CrossAttentionFusion kernel for 8 Trainium2 NeuronCores.

Math (per reference): two seq-len-1 cross-attention blocks (each reduces to
out_proj(v_proj(x)) = one fused E x E matmul), residual+LN after each, then a
4E FFN with exact-erf GELU and a final residual+LN.

Strategy:
  - Pure data parallel over the batch (16384 rows -> 2048 rows per core).
  - Feature-major ("transposed") activations on device: tiles are
    [128 features, batch] so every matmul is lhsT(=W.T chunk).T @ x.T with no
    on-device transposes. LayerNorm reductions over features run on the PE
    (ones-vector matmuls) with K=1 broadcast matmuls for mean/rstd.
  - Attention pairs are fused on the host: W1 = w_out1 @ wv1, and the biases
    b1 = w_out1 @ bv1 + b_out1 are folded into the residual inputs.
  - att1/att2 run in fp8e4m3 with DoubleRow perf mode (2 contraction rows per
    cycle); weights are host-scaled by WS and dequantized by 1/WS during the
    PSUM evacuation. Rel-err budget allows fp8 here but not in the FFN.
  - LayerNorm feature sums/sumsq also use fp8 DoubleRow (statistics tolerate
    fp8), halving the LN matmul count.
  - FFN stays bf16 (weights + activations) at full PE rate; the hidden h
    [4096 x batch] is spilled through DRAM between ffn1/ffn2 in bf16.
"""

import os
import sys

import numpy as np

sys.path.insert(0, "/opt/trn_rl_repo")

E = 1024
B = 16384
NCORES = 8
R = B // NCORES          # rows per core
CH = E // 128            # feature chunks (8)
F = 4 * E                # ffn hidden (4096)
FCH = F // 128           # ffn hidden chunks (32)
NGRP = 4                 # ffn1 weight pieces (each 1024 wide)
N = 512                  # batch tile
NT = R // N              # 4
KP = CH // 2             # fp8 DoubleRow k-pairs per E contraction (4)
WS = 64.0                # host-side fp8 weight scale
INV = 1.0 / WS

# CoreSim does not implement Gelu; tests may set KERNEL_GELU=Tanh for
# structural sim checks. Hardware always uses the real (erf) Gelu.
_GELU_FUNC = os.environ.get("KERNEL_GELU", "Gelu")
_ATT8 = os.environ.get("KERNEL_ATT8", "1") == "1"   # fp8 DoubleRow attention
_LN8 = os.environ.get("KERNEL_LN8", "1") == "1"     # fp8 DoubleRow LN sums

_RUNNER = None


def _emit_program(nc, repeats=1, phases="acd"):
    import concourse.bass as bass
    import concourse.mybir as mybir
    import concourse.tile as tile

    F32 = mybir.dt.float32
    F32R = mybir.dt.float32r
    BF16 = mybir.dt.bfloat16
    FP8 = mybir.dt.float8e4
    AF = mybir.ActivationFunctionType
    OP = mybir.AluOpType
    DR = mybir.MatmulPerfMode.DoubleRow
    ts = bass.ts

    # fp8 attention inputs
    xt8 = nc.declare_dram_parameter("xt8", [E, R], FP8, isOutput=False)
    # residuals with attention biases folded in on the host
    itres = nc.declare_dram_parameter("itres", [E, R], BF16, isOutput=False)
    xtres = nc.declare_dram_parameter("xtres", [E, R], BF16, isOutput=False)
    w1t8 = nc.declare_dram_parameter("w1t8", [E, E], FP8, isOutput=False)
    w2t8 = nc.declare_dram_parameter("w2t8", [E, E], FP8, isOutput=False)
    wf1t = nc.declare_dram_parameter("wf1t", [E, F], BF16, isOutput=False)
    wf2t = nc.declare_dram_parameter("wf2t", [F, E], BF16, isOutput=False)
    # packed per-partition params: [128, c] with [p, c] = v[c*128+p]
    bf1 = nc.declare_dram_parameter("bf1", [128, FCH], F32, isOutput=False)
    bf2 = nc.declare_dram_parameter("bf2", [128, CH], F32, isOutput=False)
    # ln params: 6 groups of CH cols: g1 be1 g2 be2 g3 be3
    lnp = nc.declare_dram_parameter("lnp", [128, 6 * CH], F32, isOutput=False)
    ones_in = nc.declare_dram_parameter("ones_in", [128, 1], F32R, isOutput=False)
    ones1_in = nc.declare_dram_parameter("ones1_in", [1, 128], F32R, isOutput=False)
    ot = nc.declare_dram_parameter("ot", [E, R], F32R, isOutput=True)

    xt8r = xt8.rearrange("(c p) r -> p c r", p=128)
    itresr = itres.rearrange("(c p) r -> p c r", p=128)
    xtresr = xtres.rearrange("(c p) r -> p c r", p=128)
    otr = ot.rearrange("(c p) r -> p c r", p=128)
    w1r = w1t8.rearrange("(c p) m -> p c m", p=128)
    w2r = w2t8.rearrange("(c p) m -> p c m", p=128)
    wf1r = wf1t.rearrange("(k p) (g j) -> g p k j", p=128, g=NGRP)
    wf2r = wf2t.rearrange("(k p) m -> p k m", p=128)

    def _mm(*a, **k):
        nc.tensor.matmul(*a, **k)

    with nc.allow_low_precision("fp8/bf16 matmul pipeline; f32 psum accum"), \
         tile.TileContext(nc) as tc:
        from contextlib import ExitStack

        with tc.tile_pool(name="dram", bufs=1, space="DRAM") as dram, \
             tc.tile_pool(name="const", bufs=1) as const:
            hbuf = dram.tile([128, FCH, R], BF16)
            cbuf = dram.tile([128, CH, R], BF16)

            bf1sb = const.tile([128, FCH], F32)
            bf2sb = const.tile([128, CH], F32)
            lnsb = const.tile([128, 6 * CH], F32)
            ones128 = const.tile([128, 1], F32R)
            ones1 = const.tile([1, 128], F32R)
            # fp8 ones pair for DoubleRow LN sums; cols padded to 16 so the
            # weight-pair stride meets the LDWEIGHTS step%16==0 rule.
            ones8 = const.tile([128, 2, 16], FP8)
            epsb = const.tile([1, 1], F32)
            zerob = const.tile([128, 1], F32)
            nc.gpsimd.dma_start(out=bf1sb[:], in_=bf1[:])
            nc.gpsimd.dma_start(out=bf2sb[:], in_=bf2[:])
            nc.gpsimd.dma_start(out=lnsb[:], in_=lnp[:])
            nc.gpsimd.dma_start(out=ones128[:], in_=ones_in[:])
            nc.gpsimd.dma_start(out=ones1[:], in_=ones1_in[:])
            nc.vector.memset(ones8[:], 1.0)
            nc.vector.memset(epsb[:], 1e-5)
            nc.vector.memset(zerob[:], 0.0)

            def layer_norm(ctx_pools, r_t, width, ln_idx, out_t):
                """LN over features of r_t [128, CH, width] -> out_t (may alias).

                Destroys r_t. ln_idx selects g/be columns in lnsb.
                ctx_pools = (sqp, stp, ps_st, ps_bc)
                PE does the feature-dim sums + broadcasts (fp8 DoubleRow when
                _LN8); ACT squares; DVE centers; ACT applies g/be.
                """
                sqp, stp, ps_st, ps_bc = ctx_pools
                g_col = lnsb[:, 2 * ln_idx * CH: (2 * ln_idx + 1) * CH]
                be_col = lnsb[:, (2 * ln_idx + 1) * CH: (2 * ln_idx + 2) * CH]
                s_ps = ps_st.tile([1, width], F32, tag="s_ps")
                q_ps = ps_st.tile([1, width], F32, tag="q_ps")
                if _LN8:
                    r8 = sqp.tile([128, CH, width], FP8, tag="r8")
                    sq8 = sqp.tile([128, CH, width], FP8, tag="sq8")
                    for m in range(CH):
                        nc.gpsimd.tensor_scalar_mul(r8[:, m, :], r_t[:, m, :], 1.0)
                        nc.scalar.activation(out=sq8[:, m, :], in_=r_t[:, m, :],
                                             func=AF.Square, bias=zerob[:])
                    for k in range(KP):
                        _mm(s_ps[:], ones8[:, :, 0:1], r8[:, 2 * k:2 * k + 2, :],
                            start=(k == 0), stop=(k == KP - 1), perf_mode=DR)
                    for k in range(KP):
                        _mm(q_ps[:], ones8[:, :, 0:1], sq8[:, 2 * k:2 * k + 2, :],
                            start=(k == 0), stop=(k == KP - 1), perf_mode=DR)
                else:
                    for m in range(CH):
                        _mm(s_ps[:], ones128[:], r_t[:, m, :],
                            start=(m == 0), stop=(m == CH - 1))
                    for m in range(CH):
                        sq = sqp.tile([128, width], F32R, tag="sq")
                        nc.scalar.activation(out=sq[:], in_=r_t[:, m, :],
                                             func=AF.Square, bias=zerob[:])
                        _mm(q_ps[:], ones128[:], sq[:],
                            start=(m == 0), stop=(m == CH - 1))
                mu_t = stp.tile([1, width], F32R, tag="mu")
                var_t = stp.tile([1, width], F32, tag="var")
                rstd_t = stp.tile([1, width], F32R, tag="rstd")
                nc.vector.tensor_scalar(out=mu_t[:], in0=s_ps[:], scalar1=1.0 / E,
                                        scalar2=None, op0=OP.mult)
                nc.vector.tensor_tensor(out=var_t[:], in0=mu_t[:], in1=mu_t[:],
                                        op=OP.mult)
                nc.vector.scalar_tensor_tensor(out=var_t[:], in0=q_ps[:],
                                               scalar=1.0 / E, in1=var_t[:],
                                               op0=OP.mult, op1=OP.subtract)
                nc.scalar.activation(out=var_t[:], in_=var_t[:], func=AF.Sqrt,
                                     bias=epsb[:])
                nc.vector.reciprocal(out=rstd_t[:], in_=var_t[:])
                mu_b = ps_bc.tile([128, width], F32, tag="mu_b")
                rstd_b = ps_bc.tile([128, width], F32, tag="rstd_b")
                _mm(mu_b[:], ones1[:], mu_t[:], start=True, stop=True)
                _mm(rstd_b[:], ones1[:], rstd_t[:], start=True, stop=True)
                for m in range(CH):
                    nc.vector.tensor_tensor(out=r_t[:, m, :], in0=r_t[:, m, :],
                                            in1=mu_b[:], op=OP.subtract)
                    nc.vector.tensor_tensor(out=r_t[:, m, :], in0=r_t[:, m, :],
                                            in1=rstd_b[:], op=OP.mult)
                    nc.scalar.activation(out=out_t[:, m, :], in_=r_t[:, m, :],
                                         func=AF.Identity,
                                         scale=g_col[:, m:m + 1],
                                         bias=be_col[:, m:m + 1])

            for rep in range(repeats):
                # ------------ Phase AB: att1+LN1+att2+LN2 -> c ------------
                if "a" in phases:
                  with ExitStack() as ab:
                      wab = ab.enter_context(tc.tile_pool(name="wab", bufs=1))
                      px = ab.enter_context(tc.tile_pool(name="px", bufs=2))
                      pit = ab.enter_context(tc.tile_pool(name="pit", bufs=2))
                      pr = ab.enter_context(tc.tile_pool(name="pr", bufs=2))
                      pimg = ab.enter_context(tc.tile_pool(name="pimg", bufs=2))
                      pi8 = ab.enter_context(tc.tile_pool(name="pi8", bufs=2))
                      pcb = ab.enter_context(tc.tile_pool(name="pcb", bufs=2))
                      sqp = ab.enter_context(tc.tile_pool(name="sqp", bufs=2))
                      stp = ab.enter_context(tc.tile_pool(name="stp", bufs=1))
                      psA = ab.enter_context(tc.tile_pool(name="psA", bufs=4, space="PSUM"))
                      ps_st = ab.enter_context(tc.tile_pool(name="ps_st", bufs=1, space="PSUM"))
                      ps_bc = ab.enter_context(tc.tile_pool(name="ps_bc", bufs=1, space="PSUM"))
                      lnpools = (sqp, stp, ps_st, ps_bc)

                      w1sb = wab.tile([128, CH, E], FP8)
                      w2sb = wab.tile([128, CH, E], FP8)
                      HCH = CH // 2
                      nc.sync.dma_start(out=w1sb[:, :HCH, :], in_=w1r[:, :HCH, :])
                      nc.sync.dma_start(out=w1sb[:, HCH:, :], in_=w1r[:, HCH:, :])
                      nc.sync.dma_start(out=w2sb[:, :HCH, :], in_=w2r[:, :HCH, :])
                      nc.sync.dma_start(out=w2sb[:, HCH:, :], in_=w2r[:, HCH:, :])

                      def attention(wsb, rhs8, res_t, out_r):
                          """out_r[m] = (wsb.T @ rhs8)[m] * INV + res[m].

                          fp8 DoubleRow over k-pairs, two m-groups of 4 so
                          matmuls start as soon as rhs chunk pairs land.
                          """
                          for mg in range(2):
                              accs = []
                              for _mi in range(4):
                                  acc_g = psA.tile([128, N], F32, tag="acc",
                                                   name=f"acc_g{_mi}")
                                  accs.append(acc_g)
                              for k in range(KP):
                                  for mi in range(4):
                                      m = mg * 4 + mi
                                      _mm(accs[mi][:],
                                          wsb[:, 2 * k:2 * k + 2, ts(m, 128)],
                                          rhs8[:, 2 * k:2 * k + 2, :],
                                          start=(k == 0), stop=(k == KP - 1),
                                          perf_mode=DR)
                              for mi in range(4):
                                  m = mg * 4 + mi
                                  nc.vector.scalar_tensor_tensor(
                                      out=out_r[:, m, :], in0=accs[mi][:],
                                      scalar=INV, in1=res_t[:, m, :],
                                      op0=OP.mult, op1=OP.add)

                      for n in range(NT):
                          sl = slice(n * N, (n + 1) * N)
                          xt8_t = px.tile([128, CH, N], FP8, tag="xt8_t")
                          nc.sync.dma_start(out=xt8_t[:, :HCH, :], in_=xt8r[:, :HCH, sl])
                          nc.sync.dma_start(out=xt8_t[:, HCH:, :], in_=xt8r[:, HCH:, sl])
                          itres_t = pit.tile([128, CH, N], BF16, tag="itres_t")
                          nc.sync.dma_start(out=itres_t[:], in_=itresr[:, :, sl])
                          xtres_t = pit.tile([128, CH, N], BF16, tag="xtres_t")
                          nc.sync.dma_start(out=xtres_t[:], in_=xtresr[:, :, sl])

                          r1 = pr.tile([128, CH, N], F32R, tag="r")
                          attention(w1sb, xt8_t, itres_t, r1)
                          imgb = pimg.tile([128, CH, N], BF16, tag="img")
                          layer_norm(lnpools, r1, N, 0, imgb)
                          img8 = pi8.tile([128, CH, N], FP8, tag="img8")
                          for m in range(CH):
                              nc.gpsimd.tensor_scalar_mul(
                                  img8[:, m, :], imgb[:, m, :], 1.0)

                          r2 = pr.tile([128, CH, N], F32R, tag="r")
                          attention(w2sb, img8, xtres_t, r2)
                          # LN2 -> txt2 (into r2), then c = txt2 + img
                          layer_norm(lnpools, r2, N, 1, r2)
                          cb = pcb.tile([128, CH, N], BF16, tag="cb")
                          for m in range(CH):
                              nc.gpsimd.tensor_tensor(
                                  out=cb[:, m, :], in0=r2[:, m, :],
                                  in1=imgb[:, m, :], op=OP.add)
                          nc.sync.dma_start(out=cbuf[:, :, sl], in_=cb[:])

                # ------------ Phase C: h = gelu(wf1 @ c + bf1) ------------
                if "c" in phases:
                  with ExitStack() as pc:
                      pcc = pc.enter_context(tc.tile_pool(name="pcc", bufs=NT))
                      pw1 = pc.enter_context(tc.tile_pool(name="pw1", bufs=2))
                      ph = pc.enter_context(tc.tile_pool(name="ph", bufs=2))
                      psC = pc.enter_context(tc.tile_pool(name="psC", bufs=4, space="PSUM"))

                      c_ts = []
                      for n in range(NT):
                          ct = pcc.tile([128, CH, N], BF16, tag="ct")
                          nc.sync.dma_start(out=ct[:], in_=cbuf[:, :, n * N:(n + 1) * N])
                          c_ts.append(ct)
                      for g in range(NGRP):
                          wg = pw1.tile([128, CH, E], BF16, tag="wg")
                          nc.sync.dma_start(out=wg[:], in_=wf1r[g])
                          for n in range(NT):
                              hst = ph.tile([128, CH, N], BF16, tag="hst")
                              for mj in range(CH):
                                  acc = psC.tile([128, N], F32, tag="accC")
                                  for k in range(CH):
                                      _mm(acc[:], wg[:, k, ts(mj, 128)],
                                          c_ts[n][:, k, :],
                                          start=(k == 0), stop=(k == CH - 1))
                                  nc.scalar.activation(
                                      hst[:, mj, :], acc[:],
                                      getattr(AF, _GELU_FUNC),
                                      bias=bf1sb[:, g * CH + mj: g * CH + mj + 1])
                              nc.sync.dma_start(
                                  out=hbuf[:, g * CH:(g + 1) * CH, n * N:(n + 1) * N],
                                  in_=hst[:])

                # ------------ Phase D: ffn2 + residual + LN3 ------------
                if "d" in phases:
                  with ExitStack() as pd:
                      phD = pd.enter_context(tc.tile_pool(name="phD", bufs=2))
                      pwm = pd.enter_context(tc.tile_pool(name="pwm", bufs=2))
                      pcD = pd.enter_context(tc.tile_pool(name="pcD", bufs=1))
                      sqpD = pd.enter_context(tc.tile_pool(name="sqpD", bufs=1))
                      stpD = pd.enter_context(tc.tile_pool(name="stpD", bufs=1))
                      psD = pd.enter_context(tc.tile_pool(name="psD", bufs=4, space="PSUM"))
                      ps_stD = pd.enter_context(tc.tile_pool(name="ps_stD", bufs=1, space="PSUM"))
                      ps_bcD = pd.enter_context(tc.tile_pool(name="ps_bcD", bufs=1, space="PSUM"))
                      lnpoolsD = (sqpD, stpD, ps_stD, ps_bcD)

                      HB = R // 2            # 1024 cols per half
                      NTH = HB // N          # 2 tiles per half
                      for half in range(2):
                          hsl = slice(half * HB, (half + 1) * HB)
                          hh = phD.tile([128, FCH, HB], BF16, tag="hh")
                          for piece in range(4):
                              pk = slice(piece * (FCH // 4), (piece + 1) * (FCH // 4))
                              nc.sync.dma_start(out=hh[:, pk, :], in_=hbuf[:, pk, hsl])
                          cres = []
                          chs = []
                          for nn in range(NTH):
                              cr = pcD.tile([128, CH, N], BF16, tag=f"cr{nn}")
                              nc.sync.dma_start(
                                  out=cr[:],
                                  in_=cbuf[:, :, half * HB + nn * N: half * HB + (nn + 1) * N])
                              cres.append(cr)
                              ch = pcD.tile([128, CH, N], F32R, tag=f"ch{nn}")
                              chs.append(ch)
                          for m in range(CH):
                              wm = pwm.tile([128, FCH, 128], BF16, tag="wm")
                              nc.sync.dma_start(out=wm[:], in_=wf2r[:, :, ts(m, 128)])
                              for nn in range(NTH):
                                  acc = psD.tile([128, N], F32, tag="accD")
                                  for k in range(FCH):
                                      _mm(acc[:], wm[:, k, :],
                                          hh[:, k, nn * N:(nn + 1) * N],
                                          start=(k == 0), stop=(k == FCH - 1))
                                  nc.vector.scalar_tensor_tensor(
                                      out=chs[nn][:, m, :], in0=acc[:],
                                      scalar=bf2sb[:, m:m + 1],
                                      in1=cres[nn][:, m, :], op0=OP.add, op1=OP.add)
                          for nn in range(NTH):
                              osl = slice(half * HB + nn * N, half * HB + (nn + 1) * N)
                              layer_norm(lnpoolsD, chs[nn], N, 2, chs[nn])
                              nc.sync.dma_start(out=otr[:, :, osl], in_=chs[nn][:])

    nc.finalize()
    return nc


def _build(repeats=1):
    from concourse import bacc

    nc = bacc.Bacc()
    return _emit_program(nc, repeats=repeats)


def _make_exec(nc, n_cores=NCORES):
    """Cached jitted SPMD executor, mirroring run_bass_via_pjrt's multi-core
    branch so repeated calls reuse the compiled NEFF."""
    import jax
    import concourse.mybir as mybir
    from concourse import bass2jax
    from jax.experimental.shard_map import shard_map
    from jax.sharding import Mesh, PartitionSpec

    bass2jax.install_neuronx_cc_hook()

    partition_name = nc.partition_id_tensor.name if nc.partition_id_tensor else None
    in_names, out_names, out_avals, zero_shapes = [], [], [], []
    for alloc in nc.m.functions[0].allocations:
        if not isinstance(alloc, mybir.MemoryLocationSet):
            continue
        name = alloc.memorylocations[0].name
        if alloc.kind == "ExternalInput":
            if name != partition_name:
                in_names.append(name)
        elif alloc.kind == "ExternalOutput":
            out_names.append(name)
            shape = tuple(alloc.tensor_shape)
            dtype = mybir.dt.np(alloc.dtype)
            out_avals.append(jax.core.ShapedArray(shape, dtype))
            zero_shapes.append((shape, dtype))
    n_params = len(in_names)
    n_outs = len(out_names)
    all_names = in_names + out_names
    if partition_name is not None:
        all_names = all_names + [partition_name]

    def _body(*args):
        operands = list(args)
        if partition_name is not None:
            operands.append(bass2jax.partition_id_tensor())
        outs = bass2jax._bass_exec_p.bind(
            *operands,
            out_avals=tuple(out_avals),
            in_names=tuple(all_names),
            out_names=tuple(out_names),
            lowering_input_output_aliases=(),
            sim_require_finite=True,
            sim_require_nnan=True,
            nc=nc,
        )
        return tuple(outs)

    devices = jax.devices()[:n_cores]
    mesh = Mesh(np.asarray(devices), ("core",))
    sharded_names = set(in_names)
    in_specs = (PartitionSpec("core"),) * (n_params + n_outs)
    out_specs = (PartitionSpec("core"),) * n_outs
    donate = tuple(range(n_params, n_params + n_outs))
    sharded = jax.jit(
        shard_map(_body, mesh=mesh, in_specs=in_specs, out_specs=out_specs,
                  check_rep=False),
        donate_argnums=donate, keep_unused=True)

    def run(in_maps):
        concat_in = [
            np.concatenate([np.asarray(in_maps[c][nm]) for c in range(n_cores)], axis=0)
            if nm in sharded_names else np.asarray(in_maps[0][nm])
            for nm in in_names
        ]
        concat_zeros = [
            np.zeros((n_cores * s[0],) + tuple(s[1:]), dt) for (s, dt) in zero_shapes
        ]
        out_arrs = sharded(*concat_in, *concat_zeros)
        out_arrs = [np.asarray(a) for a in out_arrs]
        return [
            {nm: out_arrs[i].reshape(n_cores, *out_avals[i].shape)[c]
             for i, nm in enumerate(out_names)}
            for c in range(n_cores)
        ]

    run.sharded_names = sharded_names
    run.in_names = in_names
    run.out_names = out_names
    run.sharded = sharded
    run.n_cores = n_cores
    run.out_avals = out_avals
    run.zero_shapes = zero_shapes
    run.body = _body
    run.mesh = mesh
    run.in_specs = in_specs
    run.out_specs = out_specs
    run.nc = nc
    return run


def _pack_pp(v, ch):
    """bias vector [ch*128] -> per-partition [128, ch]."""
    return np.ascontiguousarray(v.reshape(ch, 128).T.astype(np.float32))


def prepare_in_maps(img_feat, txt_feat, w_in1, b_in1, w_out1, b_out1,
                    w_in2, b_in2, w_out2, b_out2,
                    g1, be1, g2, be2, g3, be3,
                    w_ffn1, b_ffn1, w_ffn2, b_ffn2):
    import ml_dtypes
    F8 = ml_dtypes.float8_e4m3
    BF = ml_dtypes.bfloat16
    f32 = np.float32
    img = np.asarray(img_feat, f32)
    txt = np.asarray(txt_feat, f32)
    w_in1 = np.asarray(w_in1, f32); b_in1 = np.asarray(b_in1, f32)
    w_out1 = np.asarray(w_out1, f32); b_out1 = np.asarray(b_out1, f32)
    w_in2 = np.asarray(w_in2, f32); b_in2 = np.asarray(b_in2, f32)
    w_out2 = np.asarray(w_out2, f32); b_out2 = np.asarray(b_out2, f32)
    w_ffn1 = np.asarray(w_ffn1, f32); b_ffn1 = np.asarray(b_ffn1, f32)
    w_ffn2 = np.asarray(w_ffn2, f32); b_ffn2 = np.asarray(b_ffn2, f32)

    wv1 = w_in1[2 * E:]
    bv1 = b_in1[2 * E:]
    W1 = w_out1 @ wv1                      # att1 == txt @ W1.T + b1
    b1 = w_out1 @ bv1 + b_out1
    wv2 = w_in2[2 * E:]
    bv2 = b_in2[2 * E:]
    W2 = w_out2 @ wv2
    b2 = w_out2 @ bv2 + b_out2

    lnp = np.concatenate([
        _pack_pp(np.asarray(v, f32), CH)
        for v in (g1, be1, g2, be2, g3, be3)], axis=1)

    q8 = lambda x: np.clip(x, -240.0, 240.0).astype(F8)
    shared = {
        "w1t8": np.ascontiguousarray(q8(W1.T * WS)),
        "w2t8": np.ascontiguousarray(q8(W2.T * WS)),
        "wf1t": np.ascontiguousarray(w_ffn1.T.astype(BF)),
        "wf2t": np.ascontiguousarray(w_ffn2.T.astype(BF)),
        "bf1": _pack_pp(b_ffn1, FCH),
        "bf2": _pack_pp(b_ffn2, CH),
        "lnp": lnp,
        "ones_in": np.ones((128, 1), f32),
        "ones1_in": np.ones((1, 128), f32),
    }
    in_maps = []
    for c in range(NCORES):
        sh = slice(c * R, (c + 1) * R)
        m = dict(shared)
        m["xt8"] = np.ascontiguousarray(q8(txt[sh].T))
        m["itres"] = np.ascontiguousarray((img[sh].T + b1[:, None]).astype(BF))
        m["xtres"] = np.ascontiguousarray((txt[sh].T + b2[:, None]).astype(BF))
        in_maps.append(m)
    return in_maps


def get_runner():
    global _RUNNER
    if _RUNNER is None:
        nc = _build()
        _RUNNER = _make_exec(nc)
    return _RUNNER


def kernel(**inputs) -> np.ndarray:
    run = get_runner()
    in_maps = prepare_in_maps(**inputs)
    results = run(in_maps)
    out = np.empty((B, E), np.float32)
    for c in range(NCORES):
        out[c * R:(c + 1) * R] = results[c]["ot"].T
    return out


# revision 7
# speedup vs baseline: 1.7051x; 1.7051x over previous
"""CrossAttentionFusion kernel for 8 Trainium2 NeuronCores.

Math (per reference): two seq-len-1 cross-attention blocks (each reduces to
out_proj(v_proj(x)) = one fused E x E matmul), residual+LN after each, then a
4E FFN with exact-erf GELU and a final residual+LN.

Strategy:
  - Pure data parallel over the batch (16384 rows -> 2048 rows per core).
  - Feature-major ("transposed") activations on device: tiles are
    [128 features, batch] so every matmul is lhsT(=W.T chunk).T @ x.T with no
    on-device transposes. LayerNorm reductions over features run on the PE
    (ones-vector matmuls) with K=1 broadcast matmuls for mean/rstd.
  - Attention pairs are fused on the host: W1 = w_out1 @ wv1, and the biases
    b1 = w_out1 @ bv1 + b_out1 are folded into the residual inputs.
  - att1/att2 run in fp8e4m3 with DoubleRow perf mode (2 contraction rows per
    cycle); weights are host-scaled by WS and dequantized by 1/WS during the
    PSUM evacuation. Rel-err budget allows fp8 here but not in the FFN.
  - LayerNorm feature sums/sumsq also use fp8 DoubleRow (statistics tolerate
    fp8), halving the LN matmul count.
  - FFN stays bf16 (weights + activations) at full PE rate; the hidden h
    [4096 x batch] is spilled through DRAM between ffn1/ffn2 in bf16.
"""

import os
import sys

import numpy as np

sys.path.insert(0, "/opt/trn_rl_repo")

E = 1024
B = 16384
NCORES = 8
R = B // NCORES          # rows per core
CH = E // 128            # feature chunks (8)
F = 4 * E                # ffn hidden (4096)
FCH = F // 128           # ffn hidden chunks (32)
NGRP = 4                 # ffn1 weight pieces (each 1024 wide)
N = 512                  # batch tile
NT = R // N              # 4
KP = CH // 2             # fp8 DoubleRow k-pairs per E contraction (4)
WS = 64.0                # host-side fp8 weight scale
INV = 1.0 / WS

# CoreSim does not implement Gelu; tests may set KERNEL_GELU=Tanh for
# structural sim checks. Hardware always uses the real (erf) Gelu.
_GELU_FUNC = os.environ.get("KERNEL_GELU", "Gelu")
_ATT8 = os.environ.get("KERNEL_ATT8", "1") == "1"   # fp8 DoubleRow attention
_ATTDR = os.environ.get("KERNEL_ATTDR", "1") == "1"  # DoubleRow vs plain fp8
_LN8 = os.environ.get("KERNEL_LN8", "1") == "1"     # fp8 DoubleRow LN sums

_RUNNER = None


def _emit_program(nc, repeats=1, phases="acd"):
    import concourse.bass as bass
    import concourse.mybir as mybir
    import concourse.tile as tile

    F32 = mybir.dt.float32
    F32R = mybir.dt.float32r
    BF16 = mybir.dt.bfloat16
    FP8 = mybir.dt.float8e4
    AF = mybir.ActivationFunctionType
    OP = mybir.AluOpType
    DR = mybir.MatmulPerfMode.DoubleRow
    ts = bass.ts

    # fp8 attention inputs
    xt8 = nc.declare_dram_parameter("xt8", [E, R], FP8, isOutput=False)
    # residuals with attention biases folded in on the host
    itres = nc.declare_dram_parameter("itres", [E, R], BF16, isOutput=False)
    xtres = nc.declare_dram_parameter("xtres", [E, R], BF16, isOutput=False)
    w1t8 = nc.declare_dram_parameter("w1t8", [E, E], FP8, isOutput=False)
    w2t8 = nc.declare_dram_parameter("w2t8", [E, E], FP8, isOutput=False)
    wf1t = nc.declare_dram_parameter("wf1t", [E, F], BF16, isOutput=False)
    wf2t = nc.declare_dram_parameter("wf2t", [F, E], BF16, isOutput=False)
    # packed per-partition params: [128, c] with [p, c] = v[c*128+p]
    bf1 = nc.declare_dram_parameter("bf1", [128, FCH], F32, isOutput=False)
    bf2 = nc.declare_dram_parameter("bf2", [128, CH], F32, isOutput=False)
    # ln params: 6 groups of CH cols: g1 be1 g2 be2 g3 be3
    lnp = nc.declare_dram_parameter("lnp", [128, 6 * CH], F32, isOutput=False)
    ones_in = nc.declare_dram_parameter("ones_in", [128, 1], F32R, isOutput=False)
    ones1_in = nc.declare_dram_parameter("ones1_in", [1, 128], F32R, isOutput=False)
    ot = nc.declare_dram_parameter("ot", [E, R], F32R, isOutput=True)

    xt8r = xt8.rearrange("(c p) r -> p c r", p=128)
    itresr = itres.rearrange("(c p) r -> p c r", p=128)
    xtresr = xtres.rearrange("(c p) r -> p c r", p=128)
    otr = ot.rearrange("(c p) r -> p c r", p=128)
    w1r = w1t8.rearrange("(c p) m -> p c m", p=128)
    w2r = w2t8.rearrange("(c p) m -> p c m", p=128)
    wf1r = wf1t.rearrange("(k p) (g j) -> g p k j", p=128, g=NGRP)
    wf2r = wf2t.rearrange("(k p) m -> p k m", p=128)

    def _mm(*a, **k):
        nc.tensor.matmul(*a, **k)

    with nc.allow_low_precision("fp8/bf16 matmul pipeline; f32 psum accum"), \
         tile.TileContext(nc) as tc:
        from contextlib import ExitStack

        with tc.tile_pool(name="dram", bufs=1, space="DRAM") as dram, \
             tc.tile_pool(name="const", bufs=1) as const:
            hbuf = dram.tile([128, FCH, R], BF16)
            cbuf = dram.tile([128, CH, R], BF16)

            bf1sb = const.tile([128, FCH], F32)
            bf2sb = const.tile([128, CH], F32)
            lnsb = const.tile([128, 6 * CH], F32)
            ones128 = const.tile([128, 1], F32R)
            ones1 = const.tile([1, 128], F32R)
            # fp8 ones pair for DoubleRow LN sums; cols padded to 16 so the
            # weight-pair stride meets the LDWEIGHTS step%16==0 rule.
            ones8 = const.tile([128, 2, 16], FP8)
            epsb = const.tile([1, 1], F32)
            zerob = const.tile([128, 1], F32)
            nc.gpsimd.dma_start(out=bf1sb[:], in_=bf1[:])
            nc.gpsimd.dma_start(out=bf2sb[:], in_=bf2[:])
            nc.gpsimd.dma_start(out=lnsb[:], in_=lnp[:])
            nc.gpsimd.dma_start(out=ones128[:], in_=ones_in[:])
            nc.gpsimd.dma_start(out=ones1[:], in_=ones1_in[:])
            nc.vector.memset(ones8[:], 1.0)
            nc.vector.memset(epsb[:], 1e-5)
            nc.vector.memset(zerob[:], 0.0)

            def layer_norm(ctx_pools, r_t, width, ln_idx, out_t):
                """LN over features of r_t [128, CH, width] -> out_t (may alias).

                Destroys r_t. ln_idx selects g/be columns in lnsb.
                ctx_pools = (sqp, stp, ps_st, ps_bc)
                PE does the feature-dim sums + broadcasts (fp8 DoubleRow when
                _LN8); ACT squares; DVE centers; ACT applies g/be.
                """
                sqp, stp, ps_st, ps_bc = ctx_pools
                g_col = lnsb[:, 2 * ln_idx * CH: (2 * ln_idx + 1) * CH]
                be_col = lnsb[:, (2 * ln_idx + 1) * CH: (2 * ln_idx + 2) * CH]
                s_ps = ps_st.tile([1, width], F32, tag="s_ps")
                q_ps = ps_st.tile([1, width], F32, tag="q_ps")
                if _LN8:
                    r8 = sqp.tile([128, CH, width], FP8, tag="r8")
                    sq8 = sqp.tile([128, CH, width], FP8, tag="sq8")
                    for m in range(CH):
                        nc.gpsimd.tensor_scalar_mul(r8[:, m, :], r_t[:, m, :], 1.0)
                        nc.scalar.activation(out=sq8[:, m, :], in_=r_t[:, m, :],
                                             func=AF.Square, bias=zerob[:])
                    for k in range(KP):
                        _mm(s_ps[:], ones8[:, :, 0:1], r8[:, 2 * k:2 * k + 2, :],
                            start=(k == 0), stop=(k == KP - 1), perf_mode=DR)
                    for k in range(KP):
                        _mm(q_ps[:], ones8[:, :, 0:1], sq8[:, 2 * k:2 * k + 2, :],
                            start=(k == 0), stop=(k == KP - 1), perf_mode=DR)
                else:
                    for m in range(CH):
                        _mm(s_ps[:], ones128[:], r_t[:, m, :],
                            start=(m == 0), stop=(m == CH - 1))
                    for m in range(CH):
                        sq = sqp.tile([128, width], F32R, tag="sq")
                        nc.scalar.activation(out=sq[:], in_=r_t[:, m, :],
                                             func=AF.Square, bias=zerob[:])
                        _mm(q_ps[:], ones128[:], sq[:],
                            start=(m == 0), stop=(m == CH - 1))
                mu_t = stp.tile([1, width], F32R, tag="mu")
                var_t = stp.tile([1, width], F32, tag="var")
                rstd_t = stp.tile([1, width], F32R, tag="rstd")
                nc.vector.tensor_scalar(out=mu_t[:], in0=s_ps[:], scalar1=1.0 / E,
                                        scalar2=None, op0=OP.mult)
                nc.vector.tensor_tensor(out=var_t[:], in0=mu_t[:], in1=mu_t[:],
                                        op=OP.mult)
                nc.vector.scalar_tensor_tensor(out=var_t[:], in0=q_ps[:],
                                               scalar=1.0 / E, in1=var_t[:],
                                               op0=OP.mult, op1=OP.subtract)
                nc.scalar.activation(out=var_t[:], in_=var_t[:], func=AF.Sqrt,
                                     bias=epsb[:])
                nc.vector.reciprocal(out=rstd_t[:], in_=var_t[:])
                mu_b = ps_bc.tile([128, width], F32, tag="mu_b")
                rstd_b = ps_bc.tile([128, width], F32, tag="rstd_b")
                _mm(mu_b[:], ones1[:], mu_t[:], start=True, stop=True)
                _mm(rstd_b[:], ones1[:], rstd_t[:], start=True, stop=True)
                for m in range(CH):
                    nc.vector.tensor_tensor(out=r_t[:, m, :], in0=r_t[:, m, :],
                                            in1=mu_b[:], op=OP.subtract)
                    nc.vector.tensor_tensor(out=r_t[:, m, :], in0=r_t[:, m, :],
                                            in1=rstd_b[:], op=OP.mult)
                    nc.scalar.activation(out=out_t[:, m, :], in_=r_t[:, m, :],
                                         func=AF.Identity,
                                         scale=g_col[:, m:m + 1],
                                         bias=be_col[:, m:m + 1])

            for rep in range(repeats):
                # ------------ Phase AB: att1+LN1+att2+LN2 -> c ------------
                if "a" in phases:
                  with ExitStack() as ab:
                      wab = ab.enter_context(tc.tile_pool(name="wab", bufs=1))
                      px = ab.enter_context(tc.tile_pool(name="px", bufs=2))
                      pit = ab.enter_context(tc.tile_pool(name="pit", bufs=2))
                      pr = ab.enter_context(tc.tile_pool(name="pr", bufs=2))
                      pimg = ab.enter_context(tc.tile_pool(name="pimg", bufs=2))
                      pi8 = ab.enter_context(tc.tile_pool(name="pi8", bufs=2))
                      pcb = ab.enter_context(tc.tile_pool(name="pcb", bufs=2))
                      sqp = ab.enter_context(tc.tile_pool(name="sqp", bufs=2))
                      stp = ab.enter_context(tc.tile_pool(name="stp", bufs=1))
                      psA = ab.enter_context(tc.tile_pool(name="psA", bufs=4, space="PSUM"))
                      ps_st = ab.enter_context(tc.tile_pool(name="ps_st", bufs=1, space="PSUM"))
                      ps_bc = ab.enter_context(tc.tile_pool(name="ps_bc", bufs=1, space="PSUM"))
                      lnpools = (sqp, stp, ps_st, ps_bc)

                      w1sb = wab.tile([128, CH, E], FP8)
                      w2sb = wab.tile([128, CH, E], FP8)
                      HCH = CH // 2
                      nc.sync.dma_start(out=w1sb[:, :HCH, :], in_=w1r[:, :HCH, :])
                      nc.sync.dma_start(out=w1sb[:, HCH:, :], in_=w1r[:, HCH:, :])
                      nc.sync.dma_start(out=w2sb[:, :HCH, :], in_=w2r[:, :HCH, :])
                      nc.sync.dma_start(out=w2sb[:, HCH:, :], in_=w2r[:, HCH:, :])

                      def attention(wsb, rhs8, res_t, out_r):
                          """out_r[m] = (wsb.T @ rhs8)[m] * INV + res[m].

                          fp8 DoubleRow over k-pairs, two m-groups of 4 so
                          matmuls start as soon as rhs chunk pairs land.
                          """
                          for mg in range(2):
                              accs = []
                              for _mi in range(4):
                                  acc_g = psA.tile([128, N], F32, tag="acc",
                                                   name=f"acc_g{_mi}")
                                  accs.append(acc_g)
                              if _ATTDR:
                                  for k in range(KP):
                                      for mi in range(4):
                                          m = mg * 4 + mi
                                          _mm(accs[mi][:],
                                              wsb[:, 2 * k:2 * k + 2, ts(m, 128)],
                                              rhs8[:, 2 * k:2 * k + 2, :],
                                              start=(k == 0), stop=(k == KP - 1),
                                              perf_mode=DR)
                              else:
                                  for k in range(CH):
                                      for mi in range(4):
                                          m = mg * 4 + mi
                                          _mm(accs[mi][:],
                                              wsb[:, k, ts(m, 128)],
                                              rhs8[:, k, :],
                                              start=(k == 0), stop=(k == CH - 1))
                              for mi in range(4):
                                  m = mg * 4 + mi
                                  nc.vector.scalar_tensor_tensor(
                                      out=out_r[:, m, :], in0=accs[mi][:],
                                      scalar=INV, in1=res_t[:, m, :],
                                      op0=OP.mult, op1=OP.add)

                      for n in range(NT):
                          sl = slice(n * N, (n + 1) * N)
                          xt8_t = px.tile([128, CH, N], FP8, tag="xt8_t")
                          nc.sync.dma_start(out=xt8_t[:, :HCH, :], in_=xt8r[:, :HCH, sl])
                          nc.sync.dma_start(out=xt8_t[:, HCH:, :], in_=xt8r[:, HCH:, sl])
                          itres_t = pit.tile([128, CH, N], BF16, tag="itres_t")
                          nc.sync.dma_start(out=itres_t[:], in_=itresr[:, :, sl])
                          xtres_t = pit.tile([128, CH, N], BF16, tag="xtres_t")
                          nc.sync.dma_start(out=xtres_t[:], in_=xtresr[:, :, sl])

                          r1 = pr.tile([128, CH, N], F32R, tag="r")
                          attention(w1sb, xt8_t, itres_t, r1)
                          imgb = pimg.tile([128, CH, N], BF16, tag="img")
                          layer_norm(lnpools, r1, N, 0, imgb)
                          img8 = pi8.tile([128, CH, N], FP8, tag="img8")
                          for m in range(CH):
                              nc.gpsimd.tensor_scalar_mul(
                                  img8[:, m, :], imgb[:, m, :], 1.0)

                          r2 = pr.tile([128, CH, N], F32R, tag="r")
                          attention(w2sb, img8, xtres_t, r2)
                          # LN2 -> txt2 (into r2), then c = txt2 + img
                          layer_norm(lnpools, r2, N, 1, r2)
                          cb = pcb.tile([128, CH, N], BF16, tag="cb")
                          for m in range(CH):
                              nc.gpsimd.tensor_tensor(
                                  out=cb[:, m, :], in0=r2[:, m, :],
                                  in1=imgb[:, m, :], op=OP.add)
                          nc.sync.dma_start(out=cbuf[:, :, sl], in_=cb[:])

                # ------------ Phase C: h = gelu(wf1 @ c + bf1) ------------
                if "c" in phases:
                  with ExitStack() as pc:
                      pcc = pc.enter_context(tc.tile_pool(name="pcc", bufs=NT))
                      pw1 = pc.enter_context(tc.tile_pool(name="pw1", bufs=2))
                      ph = pc.enter_context(tc.tile_pool(name="ph", bufs=2))
                      psC = pc.enter_context(tc.tile_pool(name="psC", bufs=4, space="PSUM"))

                      c_ts = []
                      for n in range(NT):
                          ct = pcc.tile([128, CH, N], BF16, tag="ct")
                          nc.sync.dma_start(out=ct[:], in_=cbuf[:, :, n * N:(n + 1) * N])
                          c_ts.append(ct)
                      for g in range(NGRP):
                          wg = pw1.tile([128, CH, E], BF16, tag="wg")
                          nc.sync.dma_start(out=wg[:], in_=wf1r[g])
                          for n in range(NT):
                              hst = ph.tile([128, CH, N], BF16, tag="hst")
                              for mj in range(CH):
                                  acc = psC.tile([128, N], F32, tag="accC")
                                  for k in range(CH):
                                      _mm(acc[:], wg[:, k, ts(mj, 128)],
                                          c_ts[n][:, k, :],
                                          start=(k == 0), stop=(k == CH - 1))
                                  nc.scalar.activation(
                                      hst[:, mj, :], acc[:],
                                      getattr(AF, _GELU_FUNC),
                                      bias=bf1sb[:, g * CH + mj: g * CH + mj + 1])
                              nc.sync.dma_start(
                                  out=hbuf[:, g * CH:(g + 1) * CH, n * N:(n + 1) * N],
                                  in_=hst[:])

                # ------------ Phase D: ffn2 + residual + LN3 ------------
                if "d" in phases:
                  with ExitStack() as pd:
                      phD = pd.enter_context(tc.tile_pool(name="phD", bufs=2))
                      pwm = pd.enter_context(tc.tile_pool(name="pwm", bufs=2))
                      pcD = pd.enter_context(tc.tile_pool(name="pcD", bufs=1))
                      sqpD = pd.enter_context(tc.tile_pool(name="sqpD", bufs=1))
                      stpD = pd.enter_context(tc.tile_pool(name="stpD", bufs=1))
                      psD = pd.enter_context(tc.tile_pool(name="psD", bufs=4, space="PSUM"))
                      ps_stD = pd.enter_context(tc.tile_pool(name="ps_stD", bufs=1, space="PSUM"))
                      ps_bcD = pd.enter_context(tc.tile_pool(name="ps_bcD", bufs=1, space="PSUM"))
                      lnpoolsD = (sqpD, stpD, ps_stD, ps_bcD)

                      HB = R // 2            # 1024 cols per half
                      NTH = HB // N          # 2 tiles per half
                      for half in range(2):
                          hsl = slice(half * HB, (half + 1) * HB)
                          hh = phD.tile([128, FCH, HB], BF16, tag="hh")
                          for piece in range(4):
                              pk = slice(piece * (FCH // 4), (piece + 1) * (FCH // 4))
                              nc.sync.dma_start(out=hh[:, pk, :], in_=hbuf[:, pk, hsl])
                          cres = []
                          chs = []
                          for nn in range(NTH):
                              cr = pcD.tile([128, CH, N], BF16, tag=f"cr{nn}")
                              nc.sync.dma_start(
                                  out=cr[:],
                                  in_=cbuf[:, :, half * HB + nn * N: half * HB + (nn + 1) * N])
                              cres.append(cr)
                              ch = pcD.tile([128, CH, N], F32R, tag=f"ch{nn}")
                              chs.append(ch)
                          for m in range(CH):
                              wm = pwm.tile([128, FCH, 128], BF16, tag="wm")
                              nc.sync.dma_start(out=wm[:], in_=wf2r[:, :, ts(m, 128)])
                              for nn in range(NTH):
                                  acc = psD.tile([128, N], F32, tag="accD")
                                  for k in range(FCH):
                                      _mm(acc[:], wm[:, k, :],
                                          hh[:, k, nn * N:(nn + 1) * N],
                                          start=(k == 0), stop=(k == FCH - 1))
                                  nc.vector.scalar_tensor_tensor(
                                      out=chs[nn][:, m, :], in0=acc[:],
                                      scalar=bf2sb[:, m:m + 1],
                                      in1=cres[nn][:, m, :], op0=OP.add, op1=OP.add)
                          for nn in range(NTH):
                              osl = slice(half * HB + nn * N, half * HB + (nn + 1) * N)
                              layer_norm(lnpoolsD, chs[nn], N, 2, chs[nn])
                              nc.sync.dma_start(out=otr[:, :, osl], in_=chs[nn][:])

    nc.finalize()
    return nc


def _build(repeats=1):
    from concourse import bacc

    nc = bacc.Bacc()
    return _emit_program(nc, repeats=repeats)


def _make_exec(nc, n_cores=NCORES):
    """Cached jitted SPMD executor, mirroring run_bass_via_pjrt's multi-core
    branch so repeated calls reuse the compiled NEFF."""
    import jax
    import concourse.mybir as mybir
    from concourse import bass2jax
    from jax.experimental.shard_map import shard_map
    from jax.sharding import Mesh, PartitionSpec

    bass2jax.install_neuronx_cc_hook()

    partition_name = nc.partition_id_tensor.name if nc.partition_id_tensor else None
    in_names, out_names, out_avals, zero_shapes = [], [], [], []
    for alloc in nc.m.functions[0].allocations:
        if not isinstance(alloc, mybir.MemoryLocationSet):
            continue
        name = alloc.memorylocations[0].name
        if alloc.kind == "ExternalInput":
            if name != partition_name:
                in_names.append(name)
        elif alloc.kind == "ExternalOutput":
            out_names.append(name)
            shape = tuple(alloc.tensor_shape)
            dtype = mybir.dt.np(alloc.dtype)
            out_avals.append(jax.core.ShapedArray(shape, dtype))
            zero_shapes.append((shape, dtype))
    n_params = len(in_names)
    n_outs = len(out_names)
    all_names = in_names + out_names
    if partition_name is not None:
        all_names = all_names + [partition_name]

    def _body(*args):
        operands = list(args)
        if partition_name is not None:
            operands.append(bass2jax.partition_id_tensor())
        outs = bass2jax._bass_exec_p.bind(
            *operands,
            out_avals=tuple(out_avals),
            in_names=tuple(all_names),
            out_names=tuple(out_names),
            lowering_input_output_aliases=(),
            sim_require_finite=True,
            sim_require_nnan=True,
            nc=nc,
        )
        return tuple(outs)

    devices = jax.devices()[:n_cores]
    mesh = Mesh(np.asarray(devices), ("core",))
    sharded_names = set(in_names)
    in_specs = (PartitionSpec("core"),) * (n_params + n_outs)
    out_specs = (PartitionSpec("core"),) * n_outs
    donate = tuple(range(n_params, n_params + n_outs))
    sharded = jax.jit(
        shard_map(_body, mesh=mesh, in_specs=in_specs, out_specs=out_specs,
                  check_rep=False),
        donate_argnums=donate, keep_unused=True)

    def run(in_maps):
        concat_in = [
            np.concatenate([np.asarray(in_maps[c][nm]) for c in range(n_cores)], axis=0)
            if nm in sharded_names else np.asarray(in_maps[0][nm])
            for nm in in_names
        ]
        concat_zeros = [
            np.zeros((n_cores * s[0],) + tuple(s[1:]), dt) for (s, dt) in zero_shapes
        ]
        out_arrs = sharded(*concat_in, *concat_zeros)
        out_arrs = [np.asarray(a) for a in out_arrs]
        return [
            {nm: out_arrs[i].reshape(n_cores, *out_avals[i].shape)[c]
             for i, nm in enumerate(out_names)}
            for c in range(n_cores)
        ]

    run.sharded_names = sharded_names
    run.in_names = in_names
    run.out_names = out_names
    run.sharded = sharded
    run.n_cores = n_cores
    run.out_avals = out_avals
    run.zero_shapes = zero_shapes
    run.body = _body
    run.mesh = mesh
    run.in_specs = in_specs
    run.out_specs = out_specs
    run.nc = nc
    return run


def _pack_pp(v, ch):
    """bias vector [ch*128] -> per-partition [128, ch]."""
    return np.ascontiguousarray(v.reshape(ch, 128).T.astype(np.float32))


def prepare_in_maps(img_feat, txt_feat, w_in1, b_in1, w_out1, b_out1,
                    w_in2, b_in2, w_out2, b_out2,
                    g1, be1, g2, be2, g3, be3,
                    w_ffn1, b_ffn1, w_ffn2, b_ffn2):
    import ml_dtypes
    F8 = ml_dtypes.float8_e4m3
    BF = ml_dtypes.bfloat16
    f32 = np.float32
    img = np.asarray(img_feat, f32)
    txt = np.asarray(txt_feat, f32)
    w_in1 = np.asarray(w_in1, f32); b_in1 = np.asarray(b_in1, f32)
    w_out1 = np.asarray(w_out1, f32); b_out1 = np.asarray(b_out1, f32)
    w_in2 = np.asarray(w_in2, f32); b_in2 = np.asarray(b_in2, f32)
    w_out2 = np.asarray(w_out2, f32); b_out2 = np.asarray(b_out2, f32)
    w_ffn1 = np.asarray(w_ffn1, f32); b_ffn1 = np.asarray(b_ffn1, f32)
    w_ffn2 = np.asarray(w_ffn2, f32); b_ffn2 = np.asarray(b_ffn2, f32)

    wv1 = w_in1[2 * E:]
    bv1 = b_in1[2 * E:]
    W1 = w_out1 @ wv1                      # att1 == txt @ W1.T + b1
    b1 = w_out1 @ bv1 + b_out1
    wv2 = w_in2[2 * E:]
    bv2 = b_in2[2 * E:]
    W2 = w_out2 @ wv2
    b2 = w_out2 @ bv2 + b_out2

    lnp = np.concatenate([
        _pack_pp(np.asarray(v, f32), CH)
        for v in (g1, be1, g2, be2, g3, be3)], axis=1)

    q8 = lambda x: np.clip(x, -240.0, 240.0).astype(F8)
    shared = {
        "w1t8": np.ascontiguousarray(q8(W1.T * WS)),
        "w2t8": np.ascontiguousarray(q8(W2.T * WS)),
        "wf1t": np.ascontiguousarray(w_ffn1.T.astype(BF)),
        "wf2t": np.ascontiguousarray(w_ffn2.T.astype(BF)),
        "bf1": _pack_pp(b_ffn1, FCH),
        "bf2": _pack_pp(b_ffn2, CH),
        "lnp": lnp,
        "ones_in": np.ones((128, 1), f32),
        "ones1_in": np.ones((1, 128), f32),
    }
    in_maps = []
    for c in range(NCORES):
        sh = slice(c * R, (c + 1) * R)
        m = dict(shared)
        m["xt8"] = np.ascontiguousarray(q8(txt[sh].T))
        m["itres"] = np.ascontiguousarray((img[sh].T + b1[:, None]).astype(BF))
        m["xtres"] = np.ascontiguousarray((txt[sh].T + b2[:, None]).astype(BF))
        in_maps.append(m)
    return in_maps


def get_runner():
    global _RUNNER
    if _RUNNER is None:
        nc = _build()
        _RUNNER = _make_exec(nc)
    return _RUNNER


def kernel(**inputs) -> np.ndarray:
    run = get_runner()
    in_maps = prepare_in_maps(**inputs)
    results = run(in_maps)
    out = np.empty((B, E), np.float32)
    for c in range(NCORES):
        out[c * R:(c + 1) * R] = results[c]["ot"].T
    return out


# revision 10
# speedup vs baseline: 3.6896x; 2.1639x over previous
"""CrossAttentionFusion kernel for 8 Trainium2 NeuronCores.

Math (per reference): two seq-len-1 cross-attention blocks (each reduces to
out_proj(v_proj(x)) = one fused E x E matmul), residual+LN after each, then a
4E FFN with exact-erf GELU and a final residual+LN.

Strategy:
  - Pure data parallel over the batch (16384 rows -> 2048 rows per core).
  - Feature-major ("transposed") activations on device: tiles are
    [128 features, batch] so every matmul is lhsT(=W.T chunk).T @ x.T with no
    on-device transposes. LayerNorm reductions over features run on the PE
    (ones-vector matmuls) with K=1 broadcast matmuls for mean/rstd.
  - Attention pairs are fused on the host: W1 = w_out1 @ wv1, and the biases
    b1 = w_out1 @ bv1 + b_out1 are folded into the residual inputs.
  - att1/att2 run in fp8e4m3 with DoubleRow perf mode (2 contraction rows per
    cycle); weights are host-scaled by WS and dequantized by 1/WS during the
    PSUM evacuation. Rel-err budget allows fp8 here but not in the FFN.
  - LayerNorm feature sums/sumsq also use fp8 DoubleRow (statistics tolerate
    fp8), halving the LN matmul count.
  - FFN stays bf16 (weights + activations) at full PE rate; the hidden h
    [4096 x batch] is spilled through DRAM between ffn1/ffn2 in bf16.
"""

import os
import sys

import numpy as np

sys.path.insert(0, "/opt/trn_rl_repo")

E = 1024
B = 16384
NCORES = 8
R = B // NCORES          # rows per core
CH = E // 128            # feature chunks (8)
F = 4 * E                # ffn hidden (4096)
FCH = F // 128           # ffn hidden chunks (32)
NGRP = 4                 # ffn1 weight pieces (each 1024 wide)
N = 512                  # batch tile
NT = R // N              # 4
KP = CH // 2             # fp8 DoubleRow k-pairs per E contraction (4)
WS = 64.0                # host-side fp8 weight scale
INV = 1.0 / WS

# CoreSim does not implement Gelu; tests may set KERNEL_GELU=Tanh for
# structural sim checks. Hardware always uses the real (erf) Gelu.
_GELU_FUNC = os.environ.get("KERNEL_GELU", "Gelu")
_ATT8 = os.environ.get("KERNEL_ATT8", "1") == "1"   # fp8 DoubleRow attention
_ATTDR = os.environ.get("KERNEL_ATTDR", "1") == "1"  # DoubleRow vs plain fp8
_LN8 = os.environ.get("KERNEL_LN8", "0") == "1"     # fp8 DoubleRow LN sums

_RUNNER = None


def _emit_program(nc, repeats=1, phases="acd"):
    import concourse.bass as bass
    import concourse.mybir as mybir
    import concourse.tile as tile

    F32 = mybir.dt.float32
    F32R = mybir.dt.float32r
    BF16 = mybir.dt.bfloat16
    FP8 = mybir.dt.float8e4
    AF = mybir.ActivationFunctionType
    OP = mybir.AluOpType
    DR = mybir.MatmulPerfMode.DoubleRow
    ts = bass.ts

    # fp8 attention inputs
    xt = nc.declare_dram_parameter("xt", [E, R], F32R, isOutput=False)
    # residuals with attention biases folded in on the host
    itres = nc.declare_dram_parameter("itres", [E, R], BF16, isOutput=False)
    xtres = nc.declare_dram_parameter("xtres", [E, R], BF16, isOutput=False)
    w1t = nc.declare_dram_parameter("w1t", [E, E], F32R, isOutput=False)
    w2t = nc.declare_dram_parameter("w2t", [E, E], F32R, isOutput=False)
    wf1t = nc.declare_dram_parameter("wf1t", [E, F], BF16, isOutput=False)
    wf2t = nc.declare_dram_parameter("wf2t", [F, E], BF16, isOutput=False)
    # packed per-partition params: [128, c] with [p, c] = v[c*128+p]
    bf1 = nc.declare_dram_parameter("bf1", [128, FCH], F32, isOutput=False)
    bf2 = nc.declare_dram_parameter("bf2", [128, CH], F32, isOutput=False)
    # ln params: 6 groups of CH cols: g1 be1 g2 be2 g3 be3
    lnp = nc.declare_dram_parameter("lnp", [128, 6 * CH], F32, isOutput=False)
    ones_in = nc.declare_dram_parameter("ones_in", [128, 1], F32R, isOutput=False)
    ones1_in = nc.declare_dram_parameter("ones1_in", [1, 128], F32R, isOutput=False)
    ot = nc.declare_dram_parameter("ot", [E, R], F32R, isOutput=True)

    xtr = xt.rearrange("(c p) r -> p c r", p=128)
    itresr = itres.rearrange("(c p) r -> p c r", p=128)
    xtresr = xtres.rearrange("(c p) r -> p c r", p=128)
    otr = ot.rearrange("(c p) r -> p c r", p=128)
    w1r = w1t.rearrange("(c p) m -> p c m", p=128)
    w2r = w2t.rearrange("(c p) m -> p c m", p=128)
    wf1r = wf1t.rearrange("(k p) (g j) -> g p k j", p=128, g=NGRP)
    wf2r = wf2t.rearrange("(k p) m -> p k m", p=128)

    def _mm(*a, **k):
        nc.tensor.matmul(*a, **k)

    with nc.allow_low_precision("fp8/bf16 matmul pipeline; f32 psum accum"), \
         tile.TileContext(nc) as tc:
        from contextlib import ExitStack

        with tc.tile_pool(name="dram", bufs=1, space="DRAM") as dram, \
             tc.tile_pool(name="const", bufs=1) as const:
            hbuf = dram.tile([128, FCH, R], BF16)
            cbuf = dram.tile([128, CH, R], BF16)

            bf1sb = const.tile([128, FCH], F32)
            bf2sb = const.tile([128, CH], F32)
            lnsb = const.tile([128, 6 * CH], F32)
            ones128 = const.tile([128, 1], F32R)
            ones1 = const.tile([1, 128], F32R)
            # fp8 ones pair for DoubleRow LN sums; cols padded to 16 so the
            # weight-pair stride meets the LDWEIGHTS step%16==0 rule.
            ones8 = const.tile([128, 2, 16], FP8)
            epsb = const.tile([1, 1], F32)
            zerob = const.tile([128, 1], F32)
            nc.gpsimd.dma_start(out=bf1sb[:], in_=bf1[:])
            nc.gpsimd.dma_start(out=bf2sb[:], in_=bf2[:])
            nc.gpsimd.dma_start(out=lnsb[:], in_=lnp[:])
            nc.gpsimd.dma_start(out=ones128[:], in_=ones_in[:])
            nc.gpsimd.dma_start(out=ones1[:], in_=ones1_in[:])
            nc.vector.memset(ones8[:], 1.0)
            nc.vector.memset(epsb[:], 1e-5)
            nc.vector.memset(zerob[:], 0.0)

            def layer_norm(ctx_pools, r_t, width, ln_idx, out_t):
                """LN over features of r_t [128, CH, width] -> out_t (may alias).

                Destroys r_t. ln_idx selects g/be columns in lnsb.
                ctx_pools = (sqp, stp, ps_st, ps_bc)
                PE does the feature-dim sums + broadcasts (fp8 DoubleRow when
                _LN8); ACT squares; DVE centers; ACT applies g/be.
                """
                sqp, stp, ps_st, ps_bc = ctx_pools
                g_col = lnsb[:, 2 * ln_idx * CH: (2 * ln_idx + 1) * CH]
                be_col = lnsb[:, (2 * ln_idx + 1) * CH: (2 * ln_idx + 2) * CH]
                s_ps = ps_st.tile([1, width], F32, tag="s_ps")
                q_ps = ps_st.tile([1, width], F32, tag="q_ps")
                if _LN8:
                    r8 = sqp.tile([128, CH, width], FP8, tag="r8")
                    sq8 = sqp.tile([128, CH, width], FP8, tag="sq8")
                    for m in range(CH):
                        nc.gpsimd.tensor_scalar_mul(r8[:, m, :], r_t[:, m, :], 1.0)
                        nc.scalar.activation(out=sq8[:, m, :], in_=r_t[:, m, :],
                                             func=AF.Square, bias=zerob[:])
                    for k in range(KP):
                        _mm(s_ps[:], ones8[:, :, 0:1], r8[:, 2 * k:2 * k + 2, :],
                            start=(k == 0), stop=(k == KP - 1), perf_mode=DR)
                    for k in range(KP):
                        _mm(q_ps[:], ones8[:, :, 0:1], sq8[:, 2 * k:2 * k + 2, :],
                            start=(k == 0), stop=(k == KP - 1), perf_mode=DR)
                else:
                    for m in range(CH):
                        _mm(s_ps[:], ones128[:], r_t[:, m, :],
                            start=(m == 0), stop=(m == CH - 1))
                    for m in range(CH):
                        sq = sqp.tile([128, width], F32R, tag="sq")
                        nc.scalar.activation(out=sq[:], in_=r_t[:, m, :],
                                             func=AF.Square, bias=zerob[:])
                        _mm(q_ps[:], ones128[:], sq[:],
                            start=(m == 0), stop=(m == CH - 1))
                mu_t = stp.tile([1, width], F32R, tag="mu")
                var_t = stp.tile([1, width], F32, tag="var")
                rstd_t = stp.tile([1, width], F32R, tag="rstd")
                nc.vector.tensor_scalar(out=mu_t[:], in0=s_ps[:], scalar1=1.0 / E,
                                        scalar2=None, op0=OP.mult)
                nc.vector.tensor_tensor(out=var_t[:], in0=mu_t[:], in1=mu_t[:],
                                        op=OP.mult)
                nc.vector.scalar_tensor_tensor(out=var_t[:], in0=q_ps[:],
                                               scalar=1.0 / E, in1=var_t[:],
                                               op0=OP.mult, op1=OP.subtract)
                nc.scalar.activation(out=var_t[:], in_=var_t[:], func=AF.Sqrt,
                                     bias=epsb[:])
                nc.vector.reciprocal(out=rstd_t[:], in_=var_t[:])
                mu_b = ps_bc.tile([128, width], F32, tag="mu_b")
                rstd_b = ps_bc.tile([128, width], F32, tag="rstd_b")
                _mm(mu_b[:], ones1[:], mu_t[:], start=True, stop=True)
                _mm(rstd_b[:], ones1[:], rstd_t[:], start=True, stop=True)
                for m in range(CH):
                    nc.vector.tensor_tensor(out=r_t[:, m, :], in0=r_t[:, m, :],
                                            in1=mu_b[:], op=OP.subtract)
                    nc.vector.tensor_tensor(out=r_t[:, m, :], in0=r_t[:, m, :],
                                            in1=rstd_b[:], op=OP.mult)
                    nc.scalar.activation(out=out_t[:, m, :], in_=r_t[:, m, :],
                                         func=AF.Identity,
                                         scale=g_col[:, m:m + 1],
                                         bias=be_col[:, m:m + 1])

            for rep in range(repeats):
                # ------------ Phase AB: att1+LN1+att2+LN2 -> c ------------
                if "a" in phases:
                  with ExitStack() as ab:
                      wab = ab.enter_context(tc.tile_pool(name="wab", bufs=1))
                      px = ab.enter_context(tc.tile_pool(name="px", bufs=2))
                      pit = ab.enter_context(tc.tile_pool(name="pit", bufs=2))
                      pr = ab.enter_context(tc.tile_pool(name="pr", bufs=2))
                      pimg = ab.enter_context(tc.tile_pool(name="pimg", bufs=1))
                      pi8 = ab.enter_context(tc.tile_pool(name="pi8", bufs=2))
                      pcb = ab.enter_context(tc.tile_pool(name="pcb", bufs=2))
                      sqp = ab.enter_context(tc.tile_pool(name="sqp", bufs=2))
                      stp = ab.enter_context(tc.tile_pool(name="stp", bufs=1))
                      psA = ab.enter_context(tc.tile_pool(name="psA", bufs=4, space="PSUM"))
                      ps_st = ab.enter_context(tc.tile_pool(name="ps_st", bufs=1, space="PSUM"))
                      ps_bc = ab.enter_context(tc.tile_pool(name="ps_bc", bufs=1, space="PSUM"))
                      lnpools = (sqp, stp, ps_st, ps_bc)

                      w1sb = wab.tile([128, CH, E], F32R)
                      w2sb = wab.tile([128, CH, E], F32R)
                      HCH = CH // 2
                      nc.sync.dma_start(out=w1sb[:, :HCH, :], in_=w1r[:, :HCH, :])
                      nc.sync.dma_start(out=w1sb[:, HCH:, :], in_=w1r[:, HCH:, :])
                      nc.sync.dma_start(out=w2sb[:, :HCH, :], in_=w2r[:, :HCH, :])
                      nc.sync.dma_start(out=w2sb[:, HCH:, :], in_=w2r[:, HCH:, :])

                      def attention(wsb, rhs_t, res_t, out_r):
                          """out_r[m] = (wsb.T @ rhs)[m] + res[m] (res holds the
                          host-folded attention bias). f32r matmuls, k-major in
                          two m-groups of 4; evac is one DVE tensor_tensor."""
                          for mg in range(2):
                              accs = []
                              for _mi in range(4):
                                  acc_g = psA.tile([128, N], F32, tag="acc",
                                                   name=f"acc_g{_mi}")
                                  accs.append(acc_g)
                              for k in range(CH):
                                  for mi in range(4):
                                      m = mg * 4 + mi
                                      _mm(accs[mi][:],
                                          wsb[:, k, ts(m, 128)],
                                          rhs_t[:, k, :],
                                          start=(k == 0), stop=(k == CH - 1))
                              for mi in range(4):
                                  m = mg * 4 + mi
                                  nc.vector.tensor_tensor(
                                      out=out_r[:, m, :], in0=accs[mi][:],
                                      in1=res_t[:, m, :], op=OP.add)

                      for n in range(NT):
                          sl = slice(n * N, (n + 1) * N)
                          xt_t = px.tile([128, CH, N], F32R, tag="xt_t")
                          nc.sync.dma_start(out=xt_t[:, :HCH, :], in_=xtr[:, :HCH, sl])
                          nc.sync.dma_start(out=xt_t[:, HCH:, :], in_=xtr[:, HCH:, sl])
                          itres_t = pit.tile([128, CH, N], BF16, tag="itres_t")
                          nc.sync.dma_start(out=itres_t[:], in_=itresr[:, :, sl])
                          xtres_t = pit.tile([128, CH, N], BF16, tag="xtres_t")
                          nc.sync.dma_start(out=xtres_t[:], in_=xtresr[:, :, sl])

                          r1 = pr.tile([128, CH, N], F32R, tag="r")
                          attention(w1sb, xt_t, itres_t, r1)
                          img = pimg.tile([128, CH, N], F32R, tag="img")
                          layer_norm(lnpools, r1, N, 0, img)

                          r2 = pr.tile([128, CH, N], F32R, tag="r")
                          attention(w2sb, img, xtres_t, r2)
                          # LN2 -> txt2 (into r2), then c = txt2 + img
                          layer_norm(lnpools, r2, N, 1, r2)
                          cb = pcb.tile([128, CH, N], BF16, tag="cb")
                          for m in range(CH):
                              nc.gpsimd.tensor_tensor(
                                  out=cb[:, m, :], in0=r2[:, m, :],
                                  in1=img[:, m, :], op=OP.add)
                          nc.sync.dma_start(out=cbuf[:, :, sl], in_=cb[:])

                # ------------ Phase C: h = gelu(wf1 @ c + bf1) ------------
                if "c" in phases:
                  with ExitStack() as pc:
                      pcc = pc.enter_context(tc.tile_pool(name="pcc", bufs=NT))
                      pw1 = pc.enter_context(tc.tile_pool(name="pw1", bufs=2))
                      ph = pc.enter_context(tc.tile_pool(name="ph", bufs=2))
                      psC = pc.enter_context(tc.tile_pool(name="psC", bufs=4, space="PSUM"))

                      c_ts = []
                      for n in range(NT):
                          ct = pcc.tile([128, CH, N], BF16, tag="ct")
                          nc.sync.dma_start(out=ct[:], in_=cbuf[:, :, n * N:(n + 1) * N])
                          c_ts.append(ct)
                      for g in range(NGRP):
                          wg = pw1.tile([128, CH, E], BF16, tag="wg")
                          nc.sync.dma_start(out=wg[:], in_=wf1r[g])
                          for n in range(NT):
                              hst = ph.tile([128, CH, N], BF16, tag="hst")
                              for mj in range(CH):
                                  acc = psC.tile([128, N], F32, tag="accC")
                                  for k in range(CH):
                                      _mm(acc[:], wg[:, k, ts(mj, 128)],
                                          c_ts[n][:, k, :],
                                          start=(k == 0), stop=(k == CH - 1))
                                  nc.scalar.activation(
                                      hst[:, mj, :], acc[:],
                                      getattr(AF, _GELU_FUNC),
                                      bias=bf1sb[:, g * CH + mj: g * CH + mj + 1])
                              nc.sync.dma_start(
                                  out=hbuf[:, g * CH:(g + 1) * CH, n * N:(n + 1) * N],
                                  in_=hst[:])

                # ------------ Phase D: ffn2 + residual + LN3 ------------
                if "d" in phases:
                  with ExitStack() as pd:
                      phD = pd.enter_context(tc.tile_pool(name="phD", bufs=2))
                      pwm = pd.enter_context(tc.tile_pool(name="pwm", bufs=2))
                      pcD = pd.enter_context(tc.tile_pool(name="pcD", bufs=1))
                      sqpD = pd.enter_context(tc.tile_pool(name="sqpD", bufs=1))
                      stpD = pd.enter_context(tc.tile_pool(name="stpD", bufs=1))
                      psD = pd.enter_context(tc.tile_pool(name="psD", bufs=4, space="PSUM"))
                      ps_stD = pd.enter_context(tc.tile_pool(name="ps_stD", bufs=1, space="PSUM"))
                      ps_bcD = pd.enter_context(tc.tile_pool(name="ps_bcD", bufs=1, space="PSUM"))
                      lnpoolsD = (sqpD, stpD, ps_stD, ps_bcD)

                      HB = R // 2            # 1024 cols per half
                      NTH = HB // N          # 2 tiles per half
                      for half in range(2):
                          hsl = slice(half * HB, (half + 1) * HB)
                          hh = phD.tile([128, FCH, HB], BF16, tag="hh")
                          for piece in range(4):
                              pk = slice(piece * (FCH // 4), (piece + 1) * (FCH // 4))
                              nc.sync.dma_start(out=hh[:, pk, :], in_=hbuf[:, pk, hsl])
                          cres = []
                          chs = []
                          for nn in range(NTH):
                              cr = pcD.tile([128, CH, N], BF16, tag=f"cr{nn}")
                              nc.sync.dma_start(
                                  out=cr[:],
                                  in_=cbuf[:, :, half * HB + nn * N: half * HB + (nn + 1) * N])
                              cres.append(cr)
                              ch = pcD.tile([128, CH, N], F32R, tag=f"ch{nn}")
                              chs.append(ch)
                          for m in range(CH):
                              wm = pwm.tile([128, FCH, 128], BF16, tag="wm")
                              nc.sync.dma_start(out=wm[:], in_=wf2r[:, :, ts(m, 128)])
                              for nn in range(NTH):
                                  acc = psD.tile([128, N], F32, tag="accD")
                                  for k in range(FCH):
                                      _mm(acc[:], wm[:, k, :],
                                          hh[:, k, nn * N:(nn + 1) * N],
                                          start=(k == 0), stop=(k == FCH - 1))
                                  nc.vector.scalar_tensor_tensor(
                                      out=chs[nn][:, m, :], in0=acc[:],
                                      scalar=bf2sb[:, m:m + 1],
                                      in1=cres[nn][:, m, :], op0=OP.add, op1=OP.add)
                          for nn in range(NTH):
                              osl = slice(half * HB + nn * N, half * HB + (nn + 1) * N)
                              layer_norm(lnpoolsD, chs[nn], N, 2, chs[nn])
                              nc.sync.dma_start(out=otr[:, :, osl], in_=chs[nn][:])

    nc.finalize()
    return nc


def _build(repeats=1):
    from concourse import bacc

    nc = bacc.Bacc()
    return _emit_program(nc, repeats=repeats)


def _make_exec(nc, n_cores=NCORES):
    """Cached jitted SPMD executor, mirroring run_bass_via_pjrt's multi-core
    branch so repeated calls reuse the compiled NEFF."""
    import jax
    import concourse.mybir as mybir
    from concourse import bass2jax
    from jax.experimental.shard_map import shard_map
    from jax.sharding import Mesh, PartitionSpec

    bass2jax.install_neuronx_cc_hook()

    partition_name = nc.partition_id_tensor.name if nc.partition_id_tensor else None
    in_names, out_names, out_avals, zero_shapes = [], [], [], []
    for alloc in nc.m.functions[0].allocations:
        if not isinstance(alloc, mybir.MemoryLocationSet):
            continue
        name = alloc.memorylocations[0].name
        if alloc.kind == "ExternalInput":
            if name != partition_name:
                in_names.append(name)
        elif alloc.kind == "ExternalOutput":
            out_names.append(name)
            shape = tuple(alloc.tensor_shape)
            dtype = mybir.dt.np(alloc.dtype)
            out_avals.append(jax.core.ShapedArray(shape, dtype))
            zero_shapes.append((shape, dtype))
    n_params = len(in_names)
    n_outs = len(out_names)
    all_names = in_names + out_names
    if partition_name is not None:
        all_names = all_names + [partition_name]

    def _body(*args):
        operands = list(args)
        if partition_name is not None:
            operands.append(bass2jax.partition_id_tensor())
        outs = bass2jax._bass_exec_p.bind(
            *operands,
            out_avals=tuple(out_avals),
            in_names=tuple(all_names),
            out_names=tuple(out_names),
            lowering_input_output_aliases=(),
            sim_require_finite=True,
            sim_require_nnan=True,
            nc=nc,
        )
        return tuple(outs)

    devices = jax.devices()[:n_cores]
    mesh = Mesh(np.asarray(devices), ("core",))
    sharded_names = set(in_names)
    in_specs = (PartitionSpec("core"),) * (n_params + n_outs)
    out_specs = (PartitionSpec("core"),) * n_outs
    donate = tuple(range(n_params, n_params + n_outs))
    sharded = jax.jit(
        shard_map(_body, mesh=mesh, in_specs=in_specs, out_specs=out_specs,
                  check_rep=False),
        donate_argnums=donate, keep_unused=True)

    def run(in_maps):
        concat_in = [
            np.concatenate([np.asarray(in_maps[c][nm]) for c in range(n_cores)], axis=0)
            if nm in sharded_names else np.asarray(in_maps[0][nm])
            for nm in in_names
        ]
        concat_zeros = [
            np.zeros((n_cores * s[0],) + tuple(s[1:]), dt) for (s, dt) in zero_shapes
        ]
        out_arrs = sharded(*concat_in, *concat_zeros)
        out_arrs = [np.asarray(a) for a in out_arrs]
        return [
            {nm: out_arrs[i].reshape(n_cores, *out_avals[i].shape)[c]
             for i, nm in enumerate(out_names)}
            for c in range(n_cores)
        ]

    run.sharded_names = sharded_names
    run.in_names = in_names
    run.out_names = out_names
    run.sharded = sharded
    run.n_cores = n_cores
    run.out_avals = out_avals
    run.zero_shapes = zero_shapes
    run.body = _body
    run.mesh = mesh
    run.in_specs = in_specs
    run.out_specs = out_specs
    run.nc = nc
    return run


def _pack_pp(v, ch):
    """bias vector [ch*128] -> per-partition [128, ch]."""
    return np.ascontiguousarray(v.reshape(ch, 128).T.astype(np.float32))


def prepare_in_maps(img_feat, txt_feat, w_in1, b_in1, w_out1, b_out1,
                    w_in2, b_in2, w_out2, b_out2,
                    g1, be1, g2, be2, g3, be3,
                    w_ffn1, b_ffn1, w_ffn2, b_ffn2):
    import ml_dtypes
    F8 = ml_dtypes.float8_e4m3
    BF = ml_dtypes.bfloat16
    f32 = np.float32
    img = np.asarray(img_feat, f32)
    txt = np.asarray(txt_feat, f32)
    w_in1 = np.asarray(w_in1, f32); b_in1 = np.asarray(b_in1, f32)
    w_out1 = np.asarray(w_out1, f32); b_out1 = np.asarray(b_out1, f32)
    w_in2 = np.asarray(w_in2, f32); b_in2 = np.asarray(b_in2, f32)
    w_out2 = np.asarray(w_out2, f32); b_out2 = np.asarray(b_out2, f32)
    w_ffn1 = np.asarray(w_ffn1, f32); b_ffn1 = np.asarray(b_ffn1, f32)
    w_ffn2 = np.asarray(w_ffn2, f32); b_ffn2 = np.asarray(b_ffn2, f32)

    wv1 = w_in1[2 * E:]
    bv1 = b_in1[2 * E:]
    W1 = w_out1 @ wv1                      # att1 == txt @ W1.T + b1
    b1 = w_out1 @ bv1 + b_out1
    wv2 = w_in2[2 * E:]
    bv2 = b_in2[2 * E:]
    W2 = w_out2 @ wv2
    b2 = w_out2 @ bv2 + b_out2

    lnp = np.concatenate([
        _pack_pp(np.asarray(v, f32), CH)
        for v in (g1, be1, g2, be2, g3, be3)], axis=1)

    shared = {
        "w1t": np.ascontiguousarray(W1.T),
        "w2t": np.ascontiguousarray(W2.T),
        "wf1t": np.ascontiguousarray(w_ffn1.T.astype(BF)),
        "wf2t": np.ascontiguousarray(w_ffn2.T.astype(BF)),
        "bf1": _pack_pp(b_ffn1, FCH),
        "bf2": _pack_pp(b_ffn2, CH),
        "lnp": lnp,
        "ones_in": np.ones((128, 1), f32),
        "ones1_in": np.ones((1, 128), f32),
    }
    in_maps = []
    for c in range(NCORES):
        sh = slice(c * R, (c + 1) * R)
        m = dict(shared)
        m["xt"] = np.ascontiguousarray(txt[sh].T)
        m["itres"] = np.ascontiguousarray((img[sh].T + b1[:, None]).astype(BF))
        m["xtres"] = np.ascontiguousarray((txt[sh].T + b2[:, None]).astype(BF))
        in_maps.append(m)
    return in_maps


def get_runner():
    global _RUNNER
    if _RUNNER is None:
        nc = _build()
        _RUNNER = _make_exec(nc)
    return _RUNNER


def kernel(**inputs) -> np.ndarray:
    run = get_runner()
    in_maps = prepare_in_maps(**inputs)
    results = run(in_maps)
    out = np.empty((B, E), np.float32)
    for c in range(NCORES):
        out[c * R:(c + 1) * R] = results[c]["ot"].T
    return out


# revision 13
# speedup vs baseline: 71.5223x; 19.3847x over previous
"""CrossAttentionFusion kernel for 8 Trainium2 NeuronCores.

Math (per reference): two seq-len-1 cross-attention blocks (each reduces to
out_proj(v_proj(x)) = one fused E x E matmul), residual+LN after each, then a
4E FFN with exact-erf GELU and a final residual+LN.

Strategy:
  - Pure data parallel over the batch (16384 rows -> 2048 rows per core).
  - Feature-major ("transposed") activations on device: tiles are
    [128 features, batch] so every matmul is lhsT(=W.T chunk).T @ x.T with no
    on-device transposes. LayerNorm reductions over features run on the PE
    (ones-vector matmuls) with K=1 broadcast matmuls for mean/rstd.
  - Attention pairs are fused on the host: W1 = w_out1 @ wv1, and the biases
    b1 = w_out1 @ bv1 + b_out1 are folded into the (bf16) residual inputs, so
    each PSUM evacuation is a single DVE tensor_tensor add.
  - f32r (TF32-like fast fp32) matmuls for attention + FFN: 1 col/cycle at
    N=512, same rate as bf16. (fp8 DoubleRow was measured SLOWER here: the
    256-col LDWEIGHTS serializes; and gpsimd-cast hops stall the pipeline.)
  - bf16 for all spilled/streamed data: c between phases, FFN weights, the
    hidden h [4096 x batch] spilled through DRAM between ffn1/ffn2.
"""

import os
import sys

import numpy as np

sys.path.insert(0, "/opt/trn_rl_repo")

E = 1024
B = 16384
NCORES = 8
R = B // NCORES          # rows per core
CH = E // 128            # feature chunks (8)
F = 4 * E                # ffn hidden (4096)
FCH = F // 128           # ffn hidden chunks (32)
NGRP = 4                 # ffn1 weight pieces (each 1024 wide)
N = 512                  # batch tile
NT = R // N              # 4
KP = CH // 2             # fp8 DoubleRow k-pairs per E contraction (4)
WS = 64.0                # host-side fp8 weight scale
INV = 1.0 / WS

# CoreSim does not implement Gelu; tests may set KERNEL_GELU=Tanh for
# structural sim checks. Hardware always uses the real (erf) Gelu.
_GELU_FUNC = os.environ.get("KERNEL_GELU", "Gelu")
_ATT8 = os.environ.get("KERNEL_ATT8", "1") == "1"   # fp8 DoubleRow attention
_ATTDR = os.environ.get("KERNEL_ATTDR", "1") == "1"  # DoubleRow vs plain fp8
_LN8 = os.environ.get("KERNEL_LN8", "0") == "1"     # fp8 DoubleRow LN sums

_RUNNER = None


def _emit_program(nc, repeats=1, phases="acd"):
    import concourse.bass as bass
    import concourse.mybir as mybir
    import concourse.tile as tile

    F32 = mybir.dt.float32
    F32R = mybir.dt.float32r
    BF16 = mybir.dt.bfloat16
    FP8 = mybir.dt.float8e4
    AF = mybir.ActivationFunctionType
    OP = mybir.AluOpType
    DR = mybir.MatmulPerfMode.DoubleRow
    ts = bass.ts

    # fp8 attention inputs
    xt = nc.declare_dram_parameter("xt", [E, R], F32R, isOutput=False)
    # residuals with attention biases folded in on the host
    itres = nc.declare_dram_parameter("itres", [E, R], BF16, isOutput=False)
    xtres = nc.declare_dram_parameter("xtres", [E, R], BF16, isOutput=False)
    w1t = nc.declare_dram_parameter("w1t", [E, E], F32R, isOutput=False)
    w2t = nc.declare_dram_parameter("w2t", [E, E], F32R, isOutput=False)
    wf1t = nc.declare_dram_parameter("wf1t", [E, F], BF16, isOutput=False)
    wf2t = nc.declare_dram_parameter("wf2t", [F, E], BF16, isOutput=False)
    # packed per-partition params: [128, c] with [p, c] = v[c*128+p]
    bf1 = nc.declare_dram_parameter("bf1", [128, FCH], F32, isOutput=False)
    bf2 = nc.declare_dram_parameter("bf2", [128, CH], F32, isOutput=False)
    # ln params: 6 groups of CH cols: g1 be1 g2 be2 g3 be3
    lnp = nc.declare_dram_parameter("lnp", [128, 6 * CH], F32, isOutput=False)
    ones_in = nc.declare_dram_parameter("ones_in", [128, 1], F32R, isOutput=False)
    ones1_in = nc.declare_dram_parameter("ones1_in", [1, 128], F32R, isOutput=False)
    ot = nc.declare_dram_parameter("ot", [E, R], F32R, isOutput=True)

    xtr = xt.rearrange("(c p) r -> p c r", p=128)
    itresr = itres.rearrange("(c p) r -> p c r", p=128)
    xtresr = xtres.rearrange("(c p) r -> p c r", p=128)
    otr = ot.rearrange("(c p) r -> p c r", p=128)
    w1r = w1t.rearrange("(c p) m -> p c m", p=128)
    w2r = w2t.rearrange("(c p) m -> p c m", p=128)
    wf1r = wf1t.rearrange("(k p) (g j) -> g p k j", p=128, g=NGRP)
    wf2r = wf2t.rearrange("(k p) m -> p k m", p=128)

    def _mm(*a, **k):
        nc.tensor.matmul(*a, **k)

    with nc.allow_low_precision("fp8/bf16 matmul pipeline; f32 psum accum"), \
         tile.TileContext(nc) as tc:
        from contextlib import ExitStack

        with tc.tile_pool(name="dram", bufs=1, space="DRAM") as dram, \
             tc.tile_pool(name="const", bufs=1) as const:
            hbuf = dram.tile([128, FCH, R], BF16)
            cbuf = dram.tile([128, CH, R], BF16)

            bf1sb = const.tile([128, FCH], F32)
            bf2sb = const.tile([128, CH], F32)
            lnsb = const.tile([128, 6 * CH], F32)
            ones128 = const.tile([128, 1], F32R)
            ones1 = const.tile([1, 128], F32R)
            # fp8 ones pair for DoubleRow LN sums; cols padded to 16 so the
            # weight-pair stride meets the LDWEIGHTS step%16==0 rule.
            ones8 = const.tile([128, 2, 16], FP8)
            epsb = const.tile([1, 1], F32)
            zerob = const.tile([128, 1], F32)
            nc.gpsimd.dma_start(out=bf1sb[:], in_=bf1[:])
            nc.gpsimd.dma_start(out=bf2sb[:], in_=bf2[:])
            nc.gpsimd.dma_start(out=lnsb[:], in_=lnp[:])
            nc.gpsimd.dma_start(out=ones128[:], in_=ones_in[:])
            nc.gpsimd.dma_start(out=ones1[:], in_=ones1_in[:])
            nc.vector.memset(ones8[:], 1.0)
            nc.vector.memset(epsb[:], 1e-5)
            nc.vector.memset(zerob[:], 0.0)

            def layer_norm(ctx_pools, r_t, width, ln_idx, out_t):
                """LN over features of r_t [128, CH, width] -> out_t (may alias).

                Destroys r_t. ln_idx selects g/be columns in lnsb.
                ctx_pools = (sqp, stp, ps_st, ps_bc)
                PE does the feature-dim sums + broadcasts (fp8 DoubleRow when
                _LN8); ACT squares; DVE centers; ACT applies g/be.
                """
                sqp, stp, ps_st, ps_bc = ctx_pools
                g_col = lnsb[:, 2 * ln_idx * CH: (2 * ln_idx + 1) * CH]
                be_col = lnsb[:, (2 * ln_idx + 1) * CH: (2 * ln_idx + 2) * CH]
                s_ps = ps_st.tile([1, width], F32, tag="s_ps")
                q_ps = ps_st.tile([1, width], F32, tag="q_ps")
                if _LN8:
                    r8 = sqp.tile([128, CH, width], FP8, tag="r8")
                    sq8 = sqp.tile([128, CH, width], FP8, tag="sq8")
                    for m in range(CH):
                        nc.gpsimd.tensor_scalar_mul(r8[:, m, :], r_t[:, m, :], 1.0)
                        nc.scalar.activation(out=sq8[:, m, :], in_=r_t[:, m, :],
                                             func=AF.Square, bias=zerob[:])
                    for k in range(KP):
                        _mm(s_ps[:], ones8[:, :, 0:1], r8[:, 2 * k:2 * k + 2, :],
                            start=(k == 0), stop=(k == KP - 1), perf_mode=DR)
                    for k in range(KP):
                        _mm(q_ps[:], ones8[:, :, 0:1], sq8[:, 2 * k:2 * k + 2, :],
                            start=(k == 0), stop=(k == KP - 1), perf_mode=DR)
                else:
                    for m in range(CH):
                        _mm(s_ps[:], ones128[:], r_t[:, m, :],
                            start=(m == 0), stop=(m == CH - 1))
                    for m in range(CH):
                        sq = sqp.tile([128, width], F32R, tag="sq")
                        nc.scalar.activation(out=sq[:], in_=r_t[:, m, :],
                                             func=AF.Square, bias=zerob[:])
                        _mm(q_ps[:], ones128[:], sq[:],
                            start=(m == 0), stop=(m == CH - 1))
                mu_t = stp.tile([1, width], F32R, tag="mu")
                var_t = stp.tile([1, width], F32, tag="var")
                rstd_t = stp.tile([1, width], F32R, tag="rstd")
                nc.vector.tensor_scalar(out=mu_t[:], in0=s_ps[:], scalar1=1.0 / E,
                                        scalar2=None, op0=OP.mult)
                nc.vector.tensor_tensor(out=var_t[:], in0=mu_t[:], in1=mu_t[:],
                                        op=OP.mult)
                nc.vector.scalar_tensor_tensor(out=var_t[:], in0=q_ps[:],
                                               scalar=1.0 / E, in1=var_t[:],
                                               op0=OP.mult, op1=OP.subtract)
                nc.scalar.activation(out=var_t[:], in_=var_t[:], func=AF.Sqrt,
                                     bias=epsb[:])
                nc.vector.reciprocal(out=rstd_t[:], in_=var_t[:])
                mu_b = ps_bc.tile([128, width], F32, tag="mu_b")
                rstd_b = ps_bc.tile([128, width], F32, tag="rstd_b")
                _mm(mu_b[:], ones1[:], mu_t[:], start=True, stop=True)
                _mm(rstd_b[:], ones1[:], rstd_t[:], start=True, stop=True)
                for m in range(CH):
                    nc.vector.tensor_tensor(out=r_t[:, m, :], in0=r_t[:, m, :],
                                            in1=mu_b[:], op=OP.subtract)
                    nc.vector.tensor_tensor(out=r_t[:, m, :], in0=r_t[:, m, :],
                                            in1=rstd_b[:], op=OP.mult)
                    nc.scalar.activation(out=out_t[:, m, :], in_=r_t[:, m, :],
                                         func=AF.Identity,
                                         scale=g_col[:, m:m + 1],
                                         bias=be_col[:, m:m + 1])

            for rep in range(repeats):
                # ------------ Phase AB: att1+LN1+att2+LN2 -> c ------------
                if "a" in phases:
                  with ExitStack() as ab:
                      wab = ab.enter_context(tc.tile_pool(name="wab", bufs=1))
                      px = ab.enter_context(tc.tile_pool(name="px", bufs=2))
                      pit = ab.enter_context(tc.tile_pool(name="pit", bufs=2))
                      pxr = ab.enter_context(tc.tile_pool(name="pxr", bufs=1))
                      pr = ab.enter_context(tc.tile_pool(name="pr", bufs=2))
                      pimg = ab.enter_context(tc.tile_pool(name="pimg", bufs=2))
                      pi8 = ab.enter_context(tc.tile_pool(name="pi8", bufs=2))
                      pcb = ab.enter_context(tc.tile_pool(name="pcb", bufs=1))
                      sqp = ab.enter_context(tc.tile_pool(name="sqp", bufs=2))
                      stp = ab.enter_context(tc.tile_pool(name="stp", bufs=1))
                      psA = ab.enter_context(tc.tile_pool(name="psA", bufs=4, space="PSUM"))
                      ps_st = ab.enter_context(tc.tile_pool(name="ps_st", bufs=1, space="PSUM"))
                      ps_bc = ab.enter_context(tc.tile_pool(name="ps_bc", bufs=1, space="PSUM"))
                      lnpools = (sqp, stp, ps_st, ps_bc)

                      w1sb = wab.tile([128, CH, E], F32R)
                      w2sb = wab.tile([128, CH, E], F32R)
                      HCH = CH // 2
                      nc.sync.dma_start(out=w1sb[:, :HCH, :], in_=w1r[:, :HCH, :])
                      nc.sync.dma_start(out=w1sb[:, HCH:, :], in_=w1r[:, HCH:, :])
                      nc.sync.dma_start(out=w2sb[:, :HCH, :], in_=w2r[:, :HCH, :])
                      nc.sync.dma_start(out=w2sb[:, HCH:, :], in_=w2r[:, HCH:, :])

                      def attention(wsb, rhs_t, res_t, out_r):
                          """out_r[m] = (wsb.T @ rhs)[m] + res[m] (res holds the
                          host-folded attention bias). f32r matmuls, k-major in
                          two m-groups of 4; evac is one DVE tensor_tensor."""
                          for mg in range(2):
                              accs = []
                              for _mi in range(4):
                                  acc_g = psA.tile([128, N], F32, tag="acc",
                                                   name=f"acc_g{_mi}")
                                  accs.append(acc_g)
                              for k in range(CH):
                                  for mi in range(4):
                                      m = mg * 4 + mi
                                      _mm(accs[mi][:],
                                          wsb[:, k, ts(m, 128)],
                                          rhs_t[:, k, :],
                                          start=(k == 0), stop=(k == CH - 1))
                              for mi in range(4):
                                  m = mg * 4 + mi
                                  nc.vector.tensor_tensor(
                                      out=out_r[:, m, :], in0=accs[mi][:],
                                      in1=res_t[:, m, :], op=OP.add)

                      for n in range(NT):
                          sl = slice(n * N, (n + 1) * N)
                          xt_t = px.tile([128, CH, N], F32R, tag="xt_t")
                          nc.sync.dma_start(out=xt_t[:, :HCH, :], in_=xtr[:, :HCH, sl])
                          nc.sync.dma_start(out=xt_t[:, HCH:, :], in_=xtr[:, HCH:, sl])
                          itres_t = pit.tile([128, CH, N], BF16, tag="itres_t")
                          nc.sync.dma_start(out=itres_t[:], in_=itresr[:, :, sl])
                          xtres_t = pxr.tile([128, CH, N], BF16, tag="xtres_t")
                          nc.sync.dma_start(out=xtres_t[:], in_=xtresr[:, :, sl])

                          r1 = pr.tile([128, CH, N], F32R, tag="r")
                          attention(w1sb, xt_t, itres_t, r1)
                          img = pimg.tile([128, CH, N], F32R, tag="img")
                          layer_norm(lnpools, r1, N, 0, img)

                          r2 = pr.tile([128, CH, N], F32R, tag="r")
                          attention(w2sb, img, xtres_t, r2)
                          # LN2 -> txt2 (into r2), then c = txt2 + img
                          layer_norm(lnpools, r2, N, 1, r2)
                          cb = pcb.tile([128, CH, N], BF16, tag="cb")
                          for m in range(CH):
                              nc.vector.tensor_tensor(
                                  out=cb[:, m, :], in0=r2[:, m, :],
                                  in1=img[:, m, :], op=OP.add)
                          nc.sync.dma_start(out=cbuf[:, :, sl], in_=cb[:])

                # ------------ Phase C: h = gelu(wf1 @ c + bf1) ------------
                if "c" in phases:
                  with ExitStack() as pc:
                      pcc = pc.enter_context(tc.tile_pool(name="pcc", bufs=NT))
                      pw1 = pc.enter_context(tc.tile_pool(name="pw1", bufs=2))
                      ph = pc.enter_context(tc.tile_pool(name="ph", bufs=2))
                      psC = pc.enter_context(tc.tile_pool(name="psC", bufs=4, space="PSUM"))

                      c_ts = []
                      for n in range(NT):
                          ct = pcc.tile([128, CH, N], BF16, tag="ct")
                          nc.sync.dma_start(out=ct[:], in_=cbuf[:, :, n * N:(n + 1) * N])
                          c_ts.append(ct)
                      for g in range(NGRP):
                          wg = pw1.tile([128, CH, E], BF16, tag="wg")
                          nc.sync.dma_start(out=wg[:], in_=wf1r[g])
                          for n in range(NT):
                              hst = ph.tile([128, CH, N], BF16, tag="hst")
                              for mj in range(CH):
                                  acc = psC.tile([128, N], F32, tag="accC")
                                  for k in range(CH):
                                      _mm(acc[:], wg[:, k, ts(mj, 128)],
                                          c_ts[n][:, k, :],
                                          start=(k == 0), stop=(k == CH - 1))
                                  nc.scalar.activation(
                                      hst[:, mj, :], acc[:],
                                      getattr(AF, _GELU_FUNC),
                                      bias=bf1sb[:, g * CH + mj: g * CH + mj + 1])
                              nc.sync.dma_start(
                                  out=hbuf[:, g * CH:(g + 1) * CH, n * N:(n + 1) * N],
                                  in_=hst[:])

                # ------------ Phase D: ffn2 + residual + LN3 ------------
                if "d" in phases:
                  with ExitStack() as pd:
                      phD = pd.enter_context(tc.tile_pool(name="phD", bufs=2))
                      pwm = pd.enter_context(tc.tile_pool(name="pwm", bufs=2))
                      pcD = pd.enter_context(tc.tile_pool(name="pcD", bufs=1))
                      sqpD = pd.enter_context(tc.tile_pool(name="sqpD", bufs=1))
                      stpD = pd.enter_context(tc.tile_pool(name="stpD", bufs=1))
                      psD = pd.enter_context(tc.tile_pool(name="psD", bufs=4, space="PSUM"))
                      ps_stD = pd.enter_context(tc.tile_pool(name="ps_stD", bufs=1, space="PSUM"))
                      ps_bcD = pd.enter_context(tc.tile_pool(name="ps_bcD", bufs=1, space="PSUM"))
                      lnpoolsD = (sqpD, stpD, ps_stD, ps_bcD)

                      HB = R // 2            # 1024 cols per half
                      NTH = HB // N          # 2 tiles per half
                      for half in range(2):
                          hsl = slice(half * HB, (half + 1) * HB)
                          hh = phD.tile([128, FCH, HB], BF16, tag="hh")
                          for piece in range(4):
                              pk = slice(piece * (FCH // 4), (piece + 1) * (FCH // 4))
                              nc.sync.dma_start(out=hh[:, pk, :], in_=hbuf[:, pk, hsl])
                          cres = []
                          chs = []
                          for nn in range(NTH):
                              cr = pcD.tile([128, CH, N], BF16, tag=f"cr{nn}")
                              nc.sync.dma_start(
                                  out=cr[:],
                                  in_=cbuf[:, :, half * HB + nn * N: half * HB + (nn + 1) * N])
                              cres.append(cr)
                              ch = pcD.tile([128, CH, N], F32R, tag=f"ch{nn}")
                              chs.append(ch)
                          for m in range(CH):
                              wm = pwm.tile([128, FCH, 128], BF16, tag="wm")
                              nc.sync.dma_start(out=wm[:], in_=wf2r[:, :, ts(m, 128)])
                              for nn in range(NTH):
                                  acc = psD.tile([128, N], F32, tag="accD")
                                  for k in range(FCH):
                                      _mm(acc[:], wm[:, k, :],
                                          hh[:, k, nn * N:(nn + 1) * N],
                                          start=(k == 0), stop=(k == FCH - 1))
                                  nc.vector.scalar_tensor_tensor(
                                      out=chs[nn][:, m, :], in0=acc[:],
                                      scalar=bf2sb[:, m:m + 1],
                                      in1=cres[nn][:, m, :], op0=OP.add, op1=OP.add)
                          for nn in range(NTH):
                              osl = slice(half * HB + nn * N, half * HB + (nn + 1) * N)
                              layer_norm(lnpoolsD, chs[nn], N, 2, chs[nn])
                              nc.sync.dma_start(out=otr[:, :, osl], in_=chs[nn][:])

    nc.finalize()
    return nc


def _build(repeats=1):
    from concourse import bacc

    nc = bacc.Bacc()
    return _emit_program(nc, repeats=repeats)


def _make_exec(nc, n_cores=NCORES):
    """Cached jitted SPMD executor, mirroring run_bass_via_pjrt's multi-core
    branch so repeated calls reuse the compiled NEFF."""
    import jax
    import concourse.mybir as mybir
    from concourse import bass2jax
    from jax.experimental.shard_map import shard_map
    from jax.sharding import Mesh, PartitionSpec

    bass2jax.install_neuronx_cc_hook()

    partition_name = nc.partition_id_tensor.name if nc.partition_id_tensor else None
    in_names, out_names, out_avals, zero_shapes = [], [], [], []
    for alloc in nc.m.functions[0].allocations:
        if not isinstance(alloc, mybir.MemoryLocationSet):
            continue
        name = alloc.memorylocations[0].name
        if alloc.kind == "ExternalInput":
            if name != partition_name:
                in_names.append(name)
        elif alloc.kind == "ExternalOutput":
            out_names.append(name)
            shape = tuple(alloc.tensor_shape)
            dtype = mybir.dt.np(alloc.dtype)
            out_avals.append(jax.core.ShapedArray(shape, dtype))
            zero_shapes.append((shape, dtype))
    n_params = len(in_names)
    n_outs = len(out_names)
    all_names = in_names + out_names
    if partition_name is not None:
        all_names = all_names + [partition_name]

    def _body(*args):
        operands = list(args)
        if partition_name is not None:
            operands.append(bass2jax.partition_id_tensor())
        outs = bass2jax._bass_exec_p.bind(
            *operands,
            out_avals=tuple(out_avals),
            in_names=tuple(all_names),
            out_names=tuple(out_names),
            lowering_input_output_aliases=(),
            sim_require_finite=True,
            sim_require_nnan=True,
            nc=nc,
        )
        return tuple(outs)

    devices = jax.devices()[:n_cores]
    mesh = Mesh(np.asarray(devices), ("core",))
    sharded_names = set(in_names)
    in_specs = (PartitionSpec("core"),) * (n_params + n_outs)
    out_specs = (PartitionSpec("core"),) * n_outs
    donate = tuple(range(n_params, n_params + n_outs))
    sharded = jax.jit(
        shard_map(_body, mesh=mesh, in_specs=in_specs, out_specs=out_specs,
                  check_rep=False),
        donate_argnums=donate, keep_unused=True)

    def run(in_maps):
        concat_in = [
            np.concatenate([np.asarray(in_maps[c][nm]) for c in range(n_cores)], axis=0)
            if nm in sharded_names else np.asarray(in_maps[0][nm])
            for nm in in_names
        ]
        concat_zeros = [
            np.zeros((n_cores * s[0],) + tuple(s[1:]), dt) for (s, dt) in zero_shapes
        ]
        out_arrs = sharded(*concat_in, *concat_zeros)
        out_arrs = [np.asarray(a) for a in out_arrs]
        return [
            {nm: out_arrs[i].reshape(n_cores, *out_avals[i].shape)[c]
             for i, nm in enumerate(out_names)}
            for c in range(n_cores)
        ]

    run.sharded_names = sharded_names
    run.in_names = in_names
    run.out_names = out_names
    run.sharded = sharded
    run.n_cores = n_cores
    run.out_avals = out_avals
    run.zero_shapes = zero_shapes
    run.body = _body
    run.mesh = mesh
    run.in_specs = in_specs
    run.out_specs = out_specs
    run.nc = nc
    return run


def _pack_pp(v, ch):
    """bias vector [ch*128] -> per-partition [128, ch]."""
    return np.ascontiguousarray(v.reshape(ch, 128).T.astype(np.float32))


def prepare_in_maps(img_feat, txt_feat, w_in1, b_in1, w_out1, b_out1,
                    w_in2, b_in2, w_out2, b_out2,
                    g1, be1, g2, be2, g3, be3,
                    w_ffn1, b_ffn1, w_ffn2, b_ffn2):
    import ml_dtypes
    F8 = ml_dtypes.float8_e4m3
    BF = ml_dtypes.bfloat16
    f32 = np.float32
    img = np.asarray(img_feat, f32)
    txt = np.asarray(txt_feat, f32)
    w_in1 = np.asarray(w_in1, f32); b_in1 = np.asarray(b_in1, f32)
    w_out1 = np.asarray(w_out1, f32); b_out1 = np.asarray(b_out1, f32)
    w_in2 = np.asarray(w_in2, f32); b_in2 = np.asarray(b_in2, f32)
    w_out2 = np.asarray(w_out2, f32); b_out2 = np.asarray(b_out2, f32)
    w_ffn1 = np.asarray(w_ffn1, f32); b_ffn1 = np.asarray(b_ffn1, f32)
    w_ffn2 = np.asarray(w_ffn2, f32); b_ffn2 = np.asarray(b_ffn2, f32)

    wv1 = w_in1[2 * E:]
    bv1 = b_in1[2 * E:]
    W1 = w_out1 @ wv1                      # att1 == txt @ W1.T + b1
    b1 = w_out1 @ bv1 + b_out1
    wv2 = w_in2[2 * E:]
    bv2 = b_in2[2 * E:]
    W2 = w_out2 @ wv2
    b2 = w_out2 @ bv2 + b_out2

    lnp = np.concatenate([
        _pack_pp(np.asarray(v, f32), CH)
        for v in (g1, be1, g2, be2, g3, be3)], axis=1)

    shared = {
        "w1t": np.ascontiguousarray(W1.T),
        "w2t": np.ascontiguousarray(W2.T),
        "wf1t": np.ascontiguousarray(w_ffn1.T.astype(BF)),
        "wf2t": np.ascontiguousarray(w_ffn2.T.astype(BF)),
        "bf1": _pack_pp(b_ffn1, FCH),
        "bf2": _pack_pp(b_ffn2, CH),
        "lnp": lnp,
        "ones_in": np.ones((128, 1), f32),
        "ones1_in": np.ones((1, 128), f32),
    }
    in_maps = []
    for c in range(NCORES):
        sh = slice(c * R, (c + 1) * R)
        m = dict(shared)
        m["xt"] = np.ascontiguousarray(txt[sh].T)
        m["itres"] = np.ascontiguousarray((img[sh].T + b1[:, None]).astype(BF))
        m["xtres"] = np.ascontiguousarray((txt[sh].T + b2[:, None]).astype(BF))
        in_maps.append(m)
    return in_maps


def get_runner():
    global _RUNNER
    if _RUNNER is None:
        nc = _build()
        _RUNNER = _make_exec(nc)
    return _RUNNER


def kernel(**inputs) -> np.ndarray:
    run = get_runner()
    in_maps = prepare_in_maps(**inputs)
    results = run(in_maps)
    out = np.empty((B, E), np.float32)
    for c in range(NCORES):
        out[c * R:(c + 1) * R] = results[c]["ot"].T
    return out
